# revision 1
# baseline (speedup 1.0000x reference)
"""GAT (2-layer, PyG-style) on 8 Trainium2 NeuronCores via Bass/Tile.

v3: dst-sharded nodes+edges across 8 cores, all matmul paths bf16.
Source rows [h|es] are fetched with 1024-index dma_gather instructions
(int16 indices; per-block edges laid out lo-sources-first so the two
table views hx[0:32768] / hx[32768:] stay core-uniform).  e_dst rows
are fetched by core-local destination index from a small per-core
table (edt / edt2, 256B row stride, 16B / 2B payload) into one
SBUF-resident per-layer tensor.  Node-table row strides are 256B
multiples (dma_gather HW requirement) while payloads are exact
(raw-built instruction, validated on HW).  Per-block vector work is
batched; the segment softmax-sum runs as accumulating one-hot matmuls
in PSUM.  Layer-2 features are AllGathered between layers (bf16).
The same index/slot arrays drive both layers (identical edge plan).
"""
import os
import sys

sys.path.insert(0, "/opt/trn_rl_repo")

import numpy as np
import ml_dtypes

import concourse.bass as bass
import concourse.mybir as mybir
import concourse.tile as tile
from concourse import bacc, bass_utils, library_config
from concourse.alu_op_type import AluOpType

P = 128
NEG_SLOPE = 0.2
GMAX = 8            # max idx columns per dma_gather = 1024 idx (HW limit)
BF = ml_dtypes.bfloat16
SPLIT = 32768       # int16 idx limit for dma_gather


def wrap_idx(vals):
    """idx sequence (len%128==0, 0<=v<32768) -> wrapped [128, len//16] int16."""
    a = np.asarray(vals, np.int64)
    assert len(a) % 128 == 0 and a.min() >= 0 and a.max() < SPLIT
    w = a.reshape(-1, 16).T.astype(np.int16)
    return np.tile(w, (8, 1))


def plan(src, dst, n_nodes, n_cores):
    npc = n_nodes // n_cores
    nblk = (npc + P - 1) // P
    order = np.argsort(dst, kind="stable")
    src_s = src[order].astype(np.int64)
    dst_s = dst[order].astype(np.int64)

    per = []            # [core][block] = (src_lo, src_hi, dst_lo_lo, dst_lo_hi)
    KL = np.zeros(nblk, np.int64)
    KH = np.zeros(nblk, np.int64)
    for c in range(n_cores):
        base = c * npc
        rows = []
        for b in range(nblk):
            n0 = base + b * P
            n1 = base + min((b + 1) * P, npc)
            e0 = np.searchsorted(dst_s, n0, side="left")
            e1 = np.searchsorted(dst_s, n1, side="left")
            s = src_s[e0:e1]
            dl = dst_s[e0:e1] - base          # core-local dst
            m = s < SPLIT
            rows.append((s[m], s[~m] - SPLIT, dl[m], dl[~m]))
            KL[b] = max(KL[b], (m.sum() + P - 1) // P)
            KH[b] = max(KH[b], ((~m).sum() + P - 1) // P)
        per.append(rows)

    Kb = KL + KH
    tob = np.concatenate([[0], np.cumsum(Kb)]).astype(np.int64)
    Ttot = int(Kb.sum())
    srcw = np.zeros((n_cores, P, Ttot * 8), np.int16)
    dstw = np.zeros((n_cores, P, Ttot * 8), np.int16)
    slot = np.full((n_cores, P, Ttot), -1.0, np.float32)
    for c in range(n_cores):
        dcols = [None] * Ttot
        for b in range(nblk):
            slo, shi, dlo, dhi = per[c][b]
            kl, kh = int(KL[b]), int(KH[b])
            t0 = int(tob[b])
            n0b = b * P
            for ss, dd, K, toff in [(slo, dlo, kl, t0), (shi, dhi, kh, t0 + kl)]:
                n = len(ss)
                npad = K * P
                if npad == 0:
                    continue
                a = np.zeros(npad, np.int64)
                a[:n] = ss
                d = np.zeros(npad, np.int64)
                d[:n] = dd
                sl = np.full(npad, -1.0, np.float32)
                sl[:n] = (dd - n0b).astype(np.float32)
                srcw[c, :, toff * 8:(toff + K) * 8] = wrap_idx(a)
                slot[c, :, toff:toff + K] = sl.reshape(K, P).T
                for k in range(K):
                    dcols[toff + k] = d[k * P:(k + 1) * P]
        for g0 in range(0, Ttot, 8):
            gn = min(8, Ttot - g0)
            seq = np.concatenate(dcols[g0:g0 + gn])
            dstw[c, :, g0 * 8:(g0 + gn) * 8] = wrap_idx(seq)
    return dict(npc=npc, nblk=nblk, KL=KL, KH=KH, Kb=Kb, tob=tob, Ttot=Ttot,
                Kmax=int(Kb.max()), srcw=srcw, dstw=dstw, slot=slot)


def raw_dma_gather(nc, out_ap, in_ap, idxs_ap, num_idxs, elem_size, elem_step):
    """dma_gather (non-transpose, DRAM source) without the %256 payload
    restriction; row stride (elem_step elements) must be a 256B multiple."""
    g = nc.gpsimd
    stride_bytes = elem_step * mybir.dt.size(in_ap.dtype)
    sb256 = stride_bytes // 256
    assert stride_bytes % 256 == 0 and sb256 < 256
    _in_ap = g.lower_ap_dma(in_ap, for_custom_bir_dma=True)
    _idxs_ap = g.lower_ap(idxs_ap)
    _out_ap = g.lower_ap(out_ap)
    return g.add_instruction(
        mybir.InstDMAGatherAnt(
            name=g.bass.get_next_instruction_name(),
            ins=[*_in_ap, _idxs_ap, g.lower_val_access(g.to_reg(num_idxs))],
            outs=[_out_ap], transpose=False, num_idxs=num_idxs,
            elem_size=elem_size, stride_bytes_256=sb256, gen_mode=0,
            single_packet=True, queue_num=0, sbuf_tokens_per_rank=0,
            sbuf_free_dim_per_rank=0, sbuf_free_dim_pad_per_rank=0,
            sbuf_byte_offset=0))


def build(pl, n_nodes, cin, heads, hid, cout, n_cores):
    HC = heads * hid            # 256
    G1W = HC + heads            # 264 = [h | es] gather payload
    R1 = 384                    # hx row stride (768B)
    G2W = cout + 1              # 65  = [h2 | es2] gather payload
    R2 = 128                    # hx2 row stride (256B)
    RE = 128                    # edt row stride (256B)
    npc, nblk = pl["npc"], pl["nblk"]
    KL, KH, Kb, tob = pl["KL"], pl["KH"], pl["Kb"], pl["tob"]
    Ttot, Kmax = pl["Ttot"], pl["Kmax"]
    NT1 = (n_nodes + P - 1) // P

    nc = bacc.Bacc("TRN2")
    f32 = mybir.dt.float32
    bf16 = mybir.dt.bfloat16
    i16 = mybir.dt.int16
    Exp = mybir.ActivationFunctionType.Exp

    xT = nc.dram_tensor("xT", [cin, n_nodes], bf16, kind="ExternalInput")
    xTl = nc.dram_tensor("xTl", [cin, npc], bf16, kind="ExternalInput")
    Wx1 = nc.dram_tensor("Wx1", [cin, G1W], bf16, kind="ExternalInput")
    Wb1 = nc.dram_tensor("Wb1", [cin, heads], bf16, kind="ExternalInput")
    Wx2 = nc.dram_tensor("Wx2", [P, 2 * (G2W + 1)], bf16, kind="ExternalInput")
    b1r = nc.dram_tensor("b1r", [P, HC], f32, kind="ExternalInput")
    b2r = nc.dram_tensor("b2r", [P, cout], f32, kind="ExternalInput")
    identb = nc.dram_tensor("identb", [P, P], bf16, kind="ExternalInput")
    iota_rep = nc.dram_tensor("iota_rep", [P, Kmax * P], bf16, kind="ExternalInput")
    srcw = nc.dram_tensor("srcw", [P, Ttot * 8], i16, kind="ExternalInput")
    dstw = nc.dram_tensor("dstw", [P, Ttot * 8], i16, kind="ExternalInput")
    slotb = nc.dram_tensor("slotb", [P, Ttot], bf16, kind="ExternalInput")
    out = nc.dram_tensor("out", [npc, cout], f32, kind="ExternalOutput")

    hx = nc.dram_tensor("hx", [n_nodes, R1], bf16)
    edt = nc.dram_tensor("edt", [npc, RE], bf16)
    edt2 = nc.dram_tensor("edt2", [npc, RE], bf16)
    hx2in = nc.dram_tensor("hx2in", [npc, R2], bf16)
    hx2 = nc.dram_tensor("hx2", [n_nodes, R2], bf16, addr_space="Shared")
    hx2loc = nc.dram_tensor("hx2loc", [n_nodes, R2], bf16)

    W2C = G2W + 1               # 66 = [h2 | es2 | ed2] from the flush matmul

    with tile.TileContext(nc) as tc:
        with tc.tile_pool(name="const", bufs=1) as cp:
            nc.gpsimd.load_library(library_config.mlp)
            ident_sb = cp.tile([P, P], bf16)
            b1_sb = cp.tile([P, HC], f32)
            b2_sb = cp.tile([P, cout], f32)
            w1_sb = cp.tile([cin, G1W], bf16)
            wb1_sb = cp.tile([cin, heads], bf16)
            w2_sb = cp.tile([P, 2 * W2C], bf16)
            iota_sb = cp.tile([P, Kmax, P], bf16)
            slot_sb = cp.tile([P, Ttot], bf16)
            nc.sync.dma_start(out=ident_sb[:], in_=identb[:, :])
            nc.sync.dma_start(out=b1_sb[:], in_=b1r[:, :])
            nc.sync.dma_start(out=b2_sb[:], in_=b2r[:, :])
            nc.sync.dma_start(out=w1_sb[:], in_=Wx1[:, :])
            nc.sync.dma_start(out=wb1_sb[:], in_=Wb1[:, :])
            nc.sync.dma_start(out=w2_sb[:], in_=Wx2[:, :])
            nc.sync.dma_start(
                out=iota_sb[:, :, :],
                in_=iota_rep[:, :].rearrange("p (k q) -> p k q", q=P))
            nc.sync.dma_start(out=slot_sb[:], in_=slotb[:, :])

            # ---------- phase 1: hx = [x@W1 | es] bf16 for ALL nodes;
            # ----------          edt = x_local@Wb1 (e_dst) for LOCAL nodes
            XB = 8
            with tc.tile_pool(name="p1", bufs=3) as p1, \
                 tc.tile_pool(name="p1ps", bufs=4, space="PSUM") as p1ps:
                for j0 in range(0, NT1, XB):
                    jn = min(XB, NT1 - j0)
                    w = min(XB * P, n_nodes - j0 * P)
                    xt_sb = p1.tile([cin, XB * P], bf16, tag="xt")
                    nc.sync.dma_start(out=xt_sb[:, :w], in_=xT[:, j0 * P : j0 * P + w])
                    for k in range(jn):
                        rows = min(P, n_nodes - (j0 + k) * P)
                        ps = p1ps.tile([P, G1W], f32, tag="ps")
                        nc.tensor.matmul(out=ps[:rows, :],
                                         lhsT=xt_sb[:, k * P : k * P + rows],
                                         rhs=w1_sb[:, :], start=True, stop=True)
                        st = p1.tile([P, G1W], bf16, tag="st")
                        nc.scalar.copy(out=st[:rows, :], in_=ps[:rows, :])
                        nc.sync.dma_start(
                            out=hx[(j0 + k) * P : (j0 + k) * P + rows, 0:G1W],
                            in_=st[:rows, :])
                for j0 in range(0, nblk, XB):
                    jn = min(XB, nblk - j0)
                    w = min(XB * P, npc - j0 * P)
                    xt_sb = p1.tile([cin, XB * P], bf16, tag="xt")
                    nc.sync.dma_start(out=xt_sb[:, :w], in_=xTl[:, j0 * P : j0 * P + w])
                    for k in range(jn):
                        rows = min(P, npc - (j0 + k) * P)
                        pse = p1ps.tile([P, heads], f32, tag="pse")
                        nc.tensor.matmul(out=pse[:rows, :],
                                         lhsT=xt_sb[:, k * P : k * P + rows],
                                         rhs=wb1_sb[:, :], start=True, stop=True)
                        ste = p1.tile([P, heads], bf16, tag="ste")
                        nc.scalar.copy(out=ste[:rows, :], in_=pse[:rows, :])
                        nc.sync.dma_start(
                            out=edt[(j0 + k) * P : (j0 + k) * P + rows, 0:heads],
                            in_=ste[:rows, :])

            tc.strict_bb_all_engine_barrier()

            with tc.tile_pool(name="ps_acc", bufs=2, space="PSUM") as ps_acc, \
                 tc.tile_pool(name="ps_tr", bufs=2, space="PSUM") as ps_tr, \
                 tc.tile_pool(name="ps_small", bufs=2, space="PSUM") as ps_small, \
                 tc.tile_pool(name="ps_edp", bufs=2, space="PSUM") as ps_edp:

                def edge_layer(layer, table, RowW, gwidth, edtab, ed_w,
                               nheads, chead, flush):
                    Cm = nheads * chead
                    Racc = Cm + nheads
                    lo_end = min(SPLIT, n_nodes)
                    with tc.tile_pool(name=f"eg{layer}", bufs=3) as eg, \
                         tc.tile_pool(name=f"ew{layer}", bufs=3) as ew, \
                         tc.tile_pool(name=f"ei{layer}", bufs=3) as ei:
                        for b in range(nblk):
                            kl, kh = int(KL[b]), int(KH[b])
                            K = kl + kh
                            t0 = int(tob[b])
                            rows = min(P, npc - b * P)
                            # per-block e_dst rows: one regular strided DMA
                            edr = ew.tile([P, nheads], bf16, tag="edr")
                            if rows < P:
                                nc.vector.memset(edr[:, :], 0.0)
                            nc.sync.dma_start(
                                out=edr[:rows, :],
                                in_=edtab[b * P : b * P + rows, 0:nheads])
                            sidx = ei.tile([P, Kmax * 8], i16, tag="sidx")
                            nc.sync.dma_start(out=sidx[:, 0:K * 8],
                                              in_=srcw[:, t0 * 8:(t0 + K) * 8])
                            hxg = eg.tile([P, K, gwidth], bf16, tag="hxg")
                            for c0 in range(0, kl, GMAX):
                                ncc = min(GMAX, kl - c0)
                                raw_dma_gather(
                                    nc, hxg[:, c0:c0 + ncc, :],
                                    table[0:lo_end, 0:gwidth],
                                    sidx[:, c0 * 8:(c0 + ncc) * 8],
                                    ncc * P, gwidth, RowW)
                            for c0 in range(kl, K, GMAX):
                                ncc = min(GMAX, K - c0)
                                raw_dma_gather(
                                    nc, hxg[:, c0:c0 + ncc, :],
                                    table[SPLIT:n_nodes, 0:gwidth],
                                    sidx[:, c0 * 8:(c0 + ncc) * 8],
                                    ncc * P, gwidth, RowW)
                            mt = ew.tile([P, K, P], bf16, tag="mt")
                            nc.vector.tensor_tensor(
                                out=mt[:, :, :], in0=iota_sb[:, 0:K, :],
                                in1=slot_sb[:, t0 : t0 + K].to_broadcast([P, K, P]),
                                op=AluOpType.is_equal)
                            # e_dst per edge via transposed one-hot matmuls
                            edp = ps_edp.tile([P, Kmax * 8], f32,
                                                tag="edp", space="PSUM")
                            for k in range(K):
                                tps = ps_tr.tile([P, P], bf16, tag="trp",
                                                 space="PSUM")
                                nc.tensor.transpose(out=tps[:], in_=mt[:, k, :],
                                                    identity=ident_sb[:])
                                mtT = ew.tile([P, P], bf16, tag="mtT")
                                if k % 2 == 0:
                                    nc.scalar.copy(out=mtT[:], in_=tps[:])
                                else:
                                    nc.vector.tensor_copy(out=mtT[:], in_=tps[:])
                                nc.tensor.matmul(
                                    out=edp[:, k * nheads:(k + 1) * nheads],
                                    lhsT=mtT[:], rhs=edr[:, :],
                                    start=True, stop=True)
                            tmp = ew.tile([P, K, nheads], f32, tag="tmp")
                            nc.vector.tensor_tensor(
                                out=tmp[:, :, :], in0=hxg[:, :, Cm : Cm + nheads],
                                in1=edp[:, 0:K * nheads].rearrange(
                                    "p (k h) -> p k h", k=K), op=AluOpType.add)
                            mxe = ew.tile([P, K, nheads], f32, tag="mxe")
                            nc.vector.tensor_scalar_max(mxe[:, :, :], tmp[:, :, :], 0.0)
                            nc.vector.tensor_scalar_min(tmp[:, :, :], tmp[:, :, :], 0.0)
                            nc.vector.scalar_tensor_tensor(
                                out=tmp[:, :, :], in0=tmp[:, :, :], scalar=NEG_SLOPE,
                                in1=mxe[:, :, :], op0=AluOpType.mult, op1=AluOpType.add)
                            ex = ew.tile([P, K, nheads], bf16, tag="ex")
                            nc.scalar.activation(ex[:, :, :], tmp[:, :, :], Exp)
                            rhs = ew.tile([P, K, Racc], bf16, tag="rhs")
                            nc.vector.tensor_tensor(
                                out=rhs[:, :, 0:Cm].rearrange(
                                    "p k (h c) -> p k h c", h=nheads),
                                in0=hxg[:, :, 0:Cm].rearrange(
                                    "p k (h c) -> p k h c", h=nheads),
                                in1=ex[:, :, :].to_broadcast([P, K, nheads, chead]),
                                op=AluOpType.mult)
                            nc.vector.tensor_copy(out=rhs[:, :, Cm:Racc],
                                                  in_=ex[:, :, :])
                            acc = ps_acc.tile([P, Racc], f32, tag="acc", space="PSUM")
                            for k in range(K):
                                nc.tensor.matmul(out=acc[:, :], lhsT=mt[:, k, :],
                                                 rhs=rhs[:, k, :],
                                                 start=(k == 0), stop=(k == K - 1))
                            flush(b, acc, rows, ew)

                # ----- layer 1 flush: normalize, elu, layer-2 dense, store
                def flush1(b, acc, rows, ew):
                    recip = ew.tile([P, heads], f32, tag="recip")
                    nc.vector.reciprocal(recip[:rows, :], acc[:rows, HC : HC + heads])
                    h1 = ew.tile([P, HC], f32, tag="h1")
                    nc.vector.tensor_tensor(
                        out=h1[:rows, :].rearrange("p (h c) -> p h c", h=heads),
                        in0=acc[:rows, 0:HC].rearrange("p (h c) -> p h c", h=heads),
                        in1=recip[:rows, :].to_broadcast([rows, heads, hid]),
                        op=AluOpType.mult)
                    nc.vector.tensor_add(out=h1[:rows, :], in0=h1[:rows, :],
                                         in1=b1_sb[:rows, :])
                    mn = ew.tile([P, HC], f32, tag="mn")
                    nc.vector.tensor_scalar_min(mn[:rows, :], h1[:rows, :], 0.0)
                    nc.scalar.activation(mn[:rows, :], mn[:rows, :], Exp)
                    mx = ew.tile([P, HC], f32, tag="mx")
                    nc.vector.tensor_scalar_max(mx[:rows, :], h1[:rows, :], 0.0)
                    h1e = ew.tile([P, HC], bf16, tag="h1e")
                    nc.vector.scalar_tensor_tensor(
                        out=h1e[:rows, :], in0=mn[:rows, :], scalar=-1.0,
                        in1=mx[:rows, :], op0=AluOpType.add, op1=AluOpType.add)
                    tp = ew.tile([P, 2, P], bf16, tag="tp")
                    h2p = ps_small.tile([P, W2C], f32, tag="h2p", space="PSUM")
                    for ch in range(2):
                        tps = ps_tr.tile([P, P], bf16, tag="trp", space="PSUM")
                        nc.tensor.transpose(out=tps[:], in_=h1e[:, ch * P:(ch + 1) * P],
                                            identity=ident_sb[:])
                        nc.scalar.copy(out=tp[:, ch, :], in_=tps[:])
                        nc.tensor.matmul(
                            out=h2p[:rows, :],
                            lhsT=tp[:, ch, 0:rows],
                            rhs=w2_sb[:, ch * W2C:(ch + 1) * W2C],
                            start=(ch == 0), stop=(ch == 1))
                    st2 = ew.tile([P, R2], bf16, tag="st2")
                    nc.vector.memset(st2[:rows, G2W:R2], 0.0)
                    nc.scalar.copy(out=st2[:rows, 0:G2W], in_=h2p[:rows, 0:G2W])
                    nc.sync.dma_start(out=hx2in[b * P : b * P + rows, :],
                                      in_=st2[:rows, :])
                    se2 = ew.tile([P, 1], bf16, tag="se2")
                    nc.scalar.copy(out=se2[:rows, :], in_=h2p[:rows, G2W:W2C])
                    nc.sync.dma_start(out=edt2[b * P : b * P + rows, 0:1],
                                      in_=se2[:rows, :])

                edge_layer(1, hx, R1, G1W, edt, heads, heads, hid, flush1)

                # ----- AllGather hx2 (explicitly fenced)
                tc.strict_bb_all_engine_barrier()
                nc.gpsimd.collective_compute(
                    "AllGather", AluOpType.bypass,
                    replica_groups=[list(range(n_cores))],
                    ins=[hx2in[:, :]], outs=[hx2[:, :]])
                tc.strict_bb_all_engine_barrier()

                # copy out of Shared space (gathers can't address it)
                with tc.tile_pool(name="cphx2", bufs=3) as cph:
                    CB = 16
                    for j0 in range(0, NT1, CB):
                        rows = min(CB * P, n_nodes - j0 * P)
                        a = rows // P
                        if a:
                            t = cph.tile([P, CB, R2], bf16, tag="cp")
                            nc.sync.dma_start(
                                out=t[:, :a, :],
                                in_=hx2[j0 * P : j0 * P + a * P, :].rearrange(
                                    "(a b) c -> b a c", b=P))
                            nc.sync.dma_start(
                                out=hx2loc[j0 * P : j0 * P + a * P, :].rearrange(
                                    "(a b) c -> b a c", b=P),
                                in_=t[:, :a, :])
                        rem = rows - a * P
                        if rem:
                            t2r = cph.tile([P, R2], bf16, tag="cpr")
                            nc.sync.dma_start(
                                out=t2r[:rem, :],
                                in_=hx2[j0 * P + a * P : j0 * P + rows, :])
                            nc.sync.dma_start(
                                out=hx2loc[j0 * P + a * P : j0 * P + rows, :],
                                in_=t2r[:rem, :])
                tc.strict_bb_all_engine_barrier()

                # ----- layer-2 edge pass
                def flush2(b, acc, rows, ew):
                    recip = ew.tile([P, 1], f32, tag="recip2")
                    nc.vector.reciprocal(recip[:rows, :], acc[:rows, cout : cout + 1])
                    o = ew.tile([P, cout], f32, tag="o")
                    nc.vector.tensor_tensor(
                        out=o[:rows, :], in0=acc[:rows, 0:cout],
                        in1=recip[:rows, :].to_broadcast([rows, cout]),
                        op=AluOpType.mult)
                    nc.vector.tensor_add(out=o[:rows, :], in0=o[:rows, :],
                                         in1=b2_sb[:rows, :])
                    nc.sync.dma_start(out=out[b * P : b * P + rows, :],
                                      in_=o[:rows, :])

                edge_layer(2, hx2loc, R2, G2W, edt2, 1, 1, cout, flush2)

    nc.compile()
    return nc


def host_prep(x, edge_index, W1, a_src1, a_dst1, b1, W2, a_src2, a_dst2, b2,
              n_cores):
    """Plan the edge partition and build per-core input maps."""
    x = np.asarray(x, np.float32)
    n_nodes, cin = x.shape
    heads, hid = np.asarray(a_src1).shape
    cout = np.asarray(W2).shape[1]
    npc = n_nodes // n_cores

    loops = np.arange(n_nodes, dtype=np.int64)
    src = np.concatenate([np.asarray(edge_index[0], np.int64), loops])
    dst = np.concatenate([np.asarray(edge_index[1], np.int64), loops])
    pl = plan(src, dst, n_nodes, n_cores)

    W1 = np.asarray(W1, np.float32)
    W1h = W1.reshape(cin, heads, hid)
    Wa1 = np.einsum("khc,hc->kh", W1h, np.asarray(a_src1, np.float32))
    Wb1 = np.einsum("khc,hc->kh", W1h, np.asarray(a_dst1, np.float32))
    Wx1 = np.concatenate([W1, Wa1], axis=1).astype(BF)

    W2 = np.asarray(W2, np.float32)
    Wa2 = (W2 * np.asarray(a_src2, np.float32)).sum(1, keepdims=True)
    Wb2 = (W2 * np.asarray(a_dst2, np.float32)).sum(1, keepdims=True)
    W2e = np.concatenate([W2, Wa2, Wb2], axis=1)
    Wx2 = np.ascontiguousarray(
        np.concatenate([W2e[:P], W2e[P:]], axis=1)).astype(BF)

    xTb = np.ascontiguousarray(x.T).astype(BF)
    common = {
        "xT": xTb,
        "Wx1": Wx1,
        "Wb1": np.ascontiguousarray(Wb1).astype(BF),
        "Wx2": Wx2,
        "b1r": np.tile(np.asarray(b1, np.float32)[None, :], (P, 1)),
        "b2r": np.tile(np.asarray(b2, np.float32)[None, :], (P, 1)),
        "identb": np.eye(P, dtype=np.float32).astype(BF),
        "iota_rep": np.tile(np.arange(P, dtype=np.float32),
                            (P, pl["Kmax"])).astype(BF),
    }
    in_maps = []
    for c in range(n_cores):
        m = dict(common)
        m["xTl"] = np.ascontiguousarray(xTb[:, c * npc:(c + 1) * npc])
        m["srcw"] = np.ascontiguousarray(pl["srcw"][c])
        m["dstw"] = np.ascontiguousarray(pl["dstw"][c])
        m["slotb"] = np.ascontiguousarray(pl["slot"][c]).astype(BF)
        in_maps.append(m)
    return pl, (n_nodes, cin, heads, hid, cout), in_maps


def run_gat(x, edge_index, W1, a_src1, a_dst1, b1, W2, a_src2, a_dst2, b2,
            n_cores=8, trace=False):
    pl, (n_nodes, cin, heads, hid, cout), in_maps = host_prep(
        x, edge_index, W1, a_src1, a_dst1, b1, W2, a_src2, a_dst2, b2, n_cores)
    nc = build(pl, n_nodes, cin, heads, hid, cout, n_cores)
    res = bass_utils.run_bass_kernel_spmd(
        nc, in_maps, core_ids=list(range(n_cores)), trace=trace)
    outp = np.concatenate([res.results[c]["out"] for c in range(n_cores)], axis=0)
    return outp[:n_nodes], res


def kernel(**inputs):
    """Full-input GAT kernel: shards internally across 8 NeuronCores."""
    x = np.asarray(inputs["x"], np.float32)
    edge_index = np.asarray(inputs["edge_index"])
    outp, _ = run_gat(
        x, edge_index,
        inputs["W1"], inputs["a_src1"], inputs["a_dst1"], inputs["b1"],
        inputs["W2"], inputs["a_src2"], inputs["a_dst2"], inputs["b2"],
        n_cores=8, trace=bool(int(os.environ.get("GAT_TRACE", "0"))))
    return outp.astype(np.float32)



# revision 3
# speedup vs baseline: 1.6787x; 1.6787x over previous
"""GAT (2-layer, PyG-style) on 8 Trainium2 NeuronCores via Bass/Tile.

v4: dst-sharded nodes+edges across 8 cores.
 - dma_gather instructions round-robin over 4 SWDGE queues (4 Q7 core
   pairs emit descriptors concurrently; 3.9x emission throughput).
 - e_dst per edge via one-hot matmuls: host-precomputed fp8 one-hot
   matrices (mtT for dst->edge broadcast, mt for edge->dst segment sum)
   stream from DRAM; fp8 lhsT x bf16 rhs matmuls are exact for 0/1
   weights.  No PE transposes, no vector is_eq.
 - phase 1 stores hx rows as full contiguous 768B rows (few large DMA
   descriptors instead of 128 strided ones per block).
 - layer-2 table is gathered directly from the AllGather Shared-space
   output (no Shared->Local bounce copy).
The same index/one-hot arrays drive both layers (identical edge plan).
"""
import os
import sys

sys.path.insert(0, "/opt/trn_rl_repo")

import numpy as np
import ml_dtypes

import concourse.bass as bass
import concourse.mybir as mybir
import concourse.tile as tile
from concourse import bacc, bass_utils, library_config
from concourse.alu_op_type import AluOpType

P = 128
NEG_SLOPE = 0.2
GMAX = 8            # max idx columns per dma_gather = 1024 idx (HW limit)
BF = ml_dtypes.bfloat16
F8 = ml_dtypes.float8_e4m3
SPLIT = 32768       # int16 idx limit for dma_gather


def wrap_idx(vals):
    """idx sequence (len%128==0, 0<=v<32768) -> wrapped [128, len//16] int16."""
    a = np.asarray(vals, np.int64)
    assert len(a) % 128 == 0 and a.min() >= 0 and a.max() < SPLIT
    w = a.reshape(-1, 16).T.astype(np.int16)
    return np.tile(w, (8, 1))


def plan(src, dst, n_nodes, n_cores):
    npc = n_nodes // n_cores
    nblk = (npc + P - 1) // P
    order = np.argsort(dst, kind="stable")
    src_s = src[order].astype(np.int64)
    dst_s = dst[order].astype(np.int64)

    per = []            # [core][block] = (src_lo, src_hi, dst_lo_lo, dst_lo_hi)
    KL = np.zeros(nblk, np.int64)
    KH = np.zeros(nblk, np.int64)
    for c in range(n_cores):
        base = c * npc
        rows = []
        for b in range(nblk):
            n0 = base + b * P
            n1 = base + min((b + 1) * P, npc)
            e0 = np.searchsorted(dst_s, n0, side="left")
            e1 = np.searchsorted(dst_s, n1, side="left")
            s = src_s[e0:e1]
            dl = dst_s[e0:e1] - base          # core-local dst
            m = s < SPLIT
            rows.append((s[m], s[~m] - SPLIT, dl[m], dl[~m]))
            KL[b] = max(KL[b], (m.sum() + P - 1) // P)
            KH[b] = max(KH[b], ((~m).sum() + P - 1) // P)
        per.append(rows)

    Kb = KL + KH
    tob = np.concatenate([[0], np.cumsum(Kb)]).astype(np.int64)
    Ttot = int(Kb.sum())
    srcw = np.zeros((n_cores, P, Ttot * 8), np.int16)
    slot = np.full((n_cores, P, Ttot), -1, np.int64)
    for c in range(n_cores):
        for b in range(nblk):
            slo, shi, dlo, dhi = per[c][b]
            kl, kh = int(KL[b]), int(KH[b])
            t0 = int(tob[b])
            n0b = b * P
            for ss, dd, K, toff in [(slo, dlo, kl, t0), (shi, dhi, kh, t0 + kl)]:
                n = len(ss)
                npad = K * P
                if npad == 0:
                    continue
                a = np.zeros(npad, np.int64)
                a[:n] = ss
                sl = np.full(npad, -1, np.int64)
                sl[:n] = (dd - n0b)
                srcw[c, :, toff * 8:(toff + K) * 8] = wrap_idx(a)
                slot[c, :, toff:toff + K] = sl.reshape(K, P).T
    # one-hot matrices, fp8: mt[p, t*128+q] = (slot[p,t]==q)  (lhsT for
    # the segment-sum edge->slot matmul); mtT[q, t*128+p] = same
    # transposed (lhsT for the dst->edge e_dst broadcast matmul).
    mtb = np.zeros((n_cores, P, Ttot * P), np.uint8)
    mtTb = np.zeros((n_cores, P, Ttot * P), np.uint8)
    one = np.float32(1.0).astype(F8).view(np.uint8)
    for c in range(n_cores):
        pp, tt = np.nonzero(slot[c] >= 0)
        qq = slot[c][pp, tt]
        mtb[c, pp, tt * P + qq] = one
        mtTb[c, qq, tt * P + pp] = one
    return dict(npc=npc, nblk=nblk, KL=KL, KH=KH, Kb=Kb, tob=tob, Ttot=Ttot,
                Kmax=int(Kb.max()), srcw=srcw,
                mtb=mtb.view(F8), mtTb=mtTb.view(F8))


def raw_dma_gather(nc, out_ap, in_ap, idxs_ap, num_idxs, elem_size, elem_step,
                   queue_num):
    """dma_gather (non-transpose, DRAM source) without the %256 payload
    restriction; row stride (elem_step elements) must be a 256B multiple."""
    g = nc.gpsimd
    stride_bytes = elem_step * mybir.dt.size(in_ap.dtype)
    sb256 = stride_bytes // 256
    assert stride_bytes % 256 == 0 and sb256 < 256
    _in_ap = g.lower_ap_dma(in_ap, for_custom_bir_dma=True)
    _idxs_ap = g.lower_ap(idxs_ap)
    _out_ap = g.lower_ap(out_ap)
    return g.add_instruction(
        mybir.InstDMAGatherAnt(
            name=g.bass.get_next_instruction_name(),
            ins=[*_in_ap, _idxs_ap, g.lower_val_access(g.to_reg(num_idxs))],
            outs=[_out_ap], transpose=False, num_idxs=num_idxs,
            elem_size=elem_size, stride_bytes_256=sb256, gen_mode=0,
            single_packet=True, queue_num=queue_num, sbuf_tokens_per_rank=0,
            sbuf_free_dim_per_rank=0, sbuf_free_dim_pad_per_rank=0,
            sbuf_byte_offset=0))


def build(pl, n_nodes, cin, heads, hid, cout, n_cores):
    HC = heads * hid            # 256
    G1W = HC + heads            # 264 = [h | es] gather payload
    R1 = 384                    # hx row stride (768B)
    G2W = cout + 1              # 65  = [h2 | es2] gather payload
    R2 = 128                    # hx2 row stride (256B)
    npc, nblk = pl["npc"], pl["nblk"]
    KL, KH, Kb, tob = pl["KL"], pl["KH"], pl["Kb"], pl["tob"]
    Ttot, Kmax = pl["Ttot"], pl["Kmax"]
    NT1 = (n_nodes + P - 1) // P

    nc = bacc.Bacc("TRN2", num_swdge_queues=4)
    f32 = mybir.dt.float32
    bf16 = mybir.dt.bfloat16
    fp8 = mybir.dt.float8e4
    i16 = mybir.dt.int16
    Exp = mybir.ActivationFunctionType.Exp

    xT = nc.dram_tensor("xT", [cin, n_nodes], bf16, kind="ExternalInput")
    xTl = nc.dram_tensor("xTl", [cin, npc], bf16, kind="ExternalInput")
    Wx1 = nc.dram_tensor("Wx1", [cin, G1W], bf16, kind="ExternalInput")
    Wb1 = nc.dram_tensor("Wb1", [cin, heads], bf16, kind="ExternalInput")
    Wx2 = nc.dram_tensor("Wx2", [P, 2 * (G2W + 1)], bf16, kind="ExternalInput")
    b1r = nc.dram_tensor("b1r", [P, HC], f32, kind="ExternalInput")
    b2r = nc.dram_tensor("b2r", [P, cout], f32, kind="ExternalInput")
    identb = nc.dram_tensor("identb", [P, P], bf16, kind="ExternalInput")
    srcw = nc.dram_tensor("srcw", [P, Ttot * 8], i16, kind="ExternalInput")
    mtb = nc.dram_tensor("mtb", [P, Ttot * P], fp8, kind="ExternalInput")
    mtTb = nc.dram_tensor("mtTb", [P, Ttot * P], fp8, kind="ExternalInput")
    out = nc.dram_tensor("out", [npc, cout], f32, kind="ExternalOutput")

    hx = nc.dram_tensor("hx", [n_nodes, R1], bf16)
    edt = nc.dram_tensor("edt", [npc, heads], bf16)
    edt2 = nc.dram_tensor("edt2", [npc, 1], bf16)
    hx2in = nc.dram_tensor("hx2in", [npc, R2], bf16)
    hx2 = nc.dram_tensor("hx2", [n_nodes, R2], bf16, addr_space="Shared")

    W2C = G2W + 1               # 66 = [h2 | es2 | ed2] from the flush matmul

    qctr = [0]

    def next_q():
        q = qctr[0] & 3
        qctr[0] += 1
        return q

    with tile.TileContext(nc) as tc:
        with tc.tile_pool(name="const", bufs=1) as cp:
            nc.gpsimd.load_library(library_config.mlp)
            ident_sb = cp.tile([P, P], bf16)
            b1_sb = cp.tile([P, HC], f32)
            b2_sb = cp.tile([P, cout], f32)
            w1_sb = cp.tile([cin, G1W], bf16)
            wb1_sb = cp.tile([cin, heads], bf16)
            w2_sb = cp.tile([P, 2 * W2C], bf16)
            sidx_sb = cp.tile([P, Ttot * 8], i16)
            nc.sync.dma_start(out=ident_sb[:], in_=identb[:, :])
            nc.sync.dma_start(out=b1_sb[:], in_=b1r[:, :])
            nc.sync.dma_start(out=b2_sb[:], in_=b2r[:, :])
            nc.sync.dma_start(out=w1_sb[:], in_=Wx1[:, :])
            nc.sync.dma_start(out=wb1_sb[:], in_=Wb1[:, :])
            nc.sync.dma_start(out=w2_sb[:], in_=Wx2[:, :])
            nc.sync.dma_start(out=sidx_sb[:], in_=srcw[:, :])

            # ---------- phase 1: hx = [x@W1 | es] bf16 for ALL nodes
            # (contiguous full-row stores); edt = x_local@Wb1 for LOCAL nodes
            XB = 8
            with tc.tile_pool(name="p1", bufs=3) as p1, \
                 tc.tile_pool(name="p1ps", bufs=4, space="PSUM") as p1ps:
                for j0 in range(0, NT1, XB):
                    jn = min(XB, NT1 - j0)
                    w = min(XB * P, n_nodes - j0 * P)
                    xt_sb = p1.tile([cin, XB * P], bf16, tag="xt")
                    nc.sync.dma_start(out=xt_sb[:, :w], in_=xT[:, j0 * P : j0 * P + w])
                    st = p1.tile([P, XB, R1], bf16, tag="st")
                    for k in range(jn):
                        rows = min(P, n_nodes - (j0 + k) * P)
                        ps = p1ps.tile([P, G1W], f32, tag="ps")
                        nc.tensor.matmul(out=ps[:rows, :],
                                         lhsT=xt_sb[:, k * P : k * P + rows],
                                         rhs=w1_sb[:, :], start=True, stop=True)
                        if k % 2 == 0:
                            nc.scalar.copy(out=st[:rows, k, 0:G1W], in_=ps[:rows, :])
                        else:
                            nc.vector.tensor_copy(out=st[:rows, k, 0:G1W], in_=ps[:rows, :])
                    full = jn if w == jn * P else jn - 1
                    if full:
                        nc.sync.dma_start(
                            out=hx[j0 * P : (j0 + full) * P, :].rearrange(
                                "(a b) c -> b a c", b=P),
                            in_=st[:, 0:full, :])
                    if full < jn:
                        rows = w - full * P
                        nc.sync.dma_start(
                            out=hx[(j0 + full) * P : j0 * P + w, 0:G1W],
                            in_=st[:rows, full, 0:G1W])
                for j0 in range(0, nblk, XB):
                    jn = min(XB, nblk - j0)
                    w = min(XB * P, npc - j0 * P)
                    xt_sb = p1.tile([cin, XB * P], bf16, tag="xt")
                    nc.sync.dma_start(out=xt_sb[:, :w], in_=xTl[:, j0 * P : j0 * P + w])
                    ste = p1.tile([P, XB, heads], bf16, tag="ste")
                    for k in range(jn):
                        rows = min(P, npc - (j0 + k) * P)
                        pse = p1ps.tile([P, heads], f32, tag="pse")
                        nc.tensor.matmul(out=pse[:rows, :],
                                         lhsT=xt_sb[:, k * P : k * P + rows],
                                         rhs=wb1_sb[:, :], start=True, stop=True)
                        if k % 2 == 0:
                            nc.scalar.copy(out=ste[:rows, k, :], in_=pse[:rows, :])
                        else:
                            nc.vector.tensor_copy(out=ste[:rows, k, :], in_=pse[:rows, :])
                    full = jn if w == jn * P else jn - 1
                    if full:
                        nc.sync.dma_start(
                            out=edt[j0 * P : (j0 + full) * P, :].rearrange(
                                "(a b) c -> b a c", b=P),
                            in_=ste[:, 0:full, :])
                    if full < jn:
                        rows = w - full * P
                        nc.sync.dma_start(
                            out=edt[(j0 + full) * P : j0 * P + w, :],
                            in_=ste[:rows, full, :])

            tc.strict_bb_all_engine_barrier()

            with tc.tile_pool(name="ps_acc", bufs=2, space="PSUM") as ps_acc, \
                 tc.tile_pool(name="ps_tr", bufs=2, space="PSUM") as ps_tr, \
                 tc.tile_pool(name="ps_small", bufs=2, space="PSUM") as ps_small, \
                 tc.tile_pool(name="ps_edp", bufs=2, space="PSUM") as ps_edp:

                def edge_layer(layer, table, RowW, gwidth, edtab, nheads,
                               chead, flush):
                    Cm = nheads * chead
                    Racc = Cm + nheads
                    lo_end = min(SPLIT, n_nodes)
                    with tc.tile_pool(name=f"eg{layer}", bufs=4) as eg, \
                         tc.tile_pool(name=f"ew{layer}", bufs=3) as ew, \
                         tc.tile_pool(name=f"em{layer}", bufs=3) as em:
                        for b in range(nblk):
                            kl, kh = int(KL[b]), int(KH[b])
                            K = kl + kh
                            t0 = int(tob[b])
                            rows = min(P, npc - b * P)
                            # per-block e_dst rows: one contiguous DMA
                            edr = ew.tile([P, nheads], bf16, tag="edr")
                            if rows < P:
                                nc.vector.memset(edr[:, :], 0.0)
                            nc.sync.dma_start(
                                out=edr[:rows, :],
                                in_=edtab[b * P : b * P + rows, :])
                            # one-hot pair for this block (fp8)
                            mt = em.tile([P, K, P], fp8, tag="mt")
                            nc.sync.dma_start(
                                out=mt[:, :, :],
                                in_=mtb[:, t0 * P:(t0 + K) * P].rearrange(
                                    "p (k q) -> p k q", q=P))
                            mtT = em.tile([P, K, P], fp8, tag="mtT")
                            nc.sync.dma_start(
                                out=mtT[:, :, :],
                                in_=mtTb[:, t0 * P:(t0 + K) * P].rearrange(
                                    "q (k p) -> q k p", p=P))
                            hxg = eg.tile([P, K, gwidth], bf16, tag="hxg")
                            for c0 in range(0, kl, GMAX):
                                ncc = min(GMAX, kl - c0)
                                raw_dma_gather(
                                    nc, hxg[:, c0:c0 + ncc, :],
                                    table[0:lo_end, 0:gwidth],
                                    sidx_sb[:, (t0 + c0) * 8:(t0 + c0 + ncc) * 8],
                                    ncc * P, gwidth, RowW, next_q())
                            for c0 in range(kl, K, GMAX):
                                ncc = min(GMAX, K - c0)
                                raw_dma_gather(
                                    nc, hxg[:, c0:c0 + ncc, :],
                                    table[SPLIT:n_nodes, 0:gwidth],
                                    sidx_sb[:, (t0 + c0) * 8:(t0 + c0 + ncc) * 8],
                                    ncc * P, gwidth, RowW, next_q())
                            # e_dst per edge: edp[p, k*nh:] = edr[slot[p,k], :]
                            edp = ps_edp.tile([P, Kmax * nheads], f32,
                                              tag="edp", space="PSUM")
                            for k in range(K):
                                nc.tensor.matmul(
                                    out=edp[:, k * nheads:(k + 1) * nheads],
                                    lhsT=mtT[:, k, :], rhs=edr[:, :],
                                    start=True, stop=True)
                            tmp = ew.tile([P, K, nheads], f32, tag="tmp")
                            nc.vector.tensor_tensor(
                                out=tmp[:, :, :], in0=hxg[:, :, Cm : Cm + nheads],
                                in1=edp[:, 0:K * nheads].rearrange(
                                    "p (k h) -> p k h", k=K), op=AluOpType.add)
                            mxe = ew.tile([P, K, nheads], f32, tag="mxe")
                            nc.vector.tensor_scalar_max(mxe[:, :, :], tmp[:, :, :], 0.0)
                            nc.vector.tensor_scalar_min(tmp[:, :, :], tmp[:, :, :], 0.0)
                            nc.vector.scalar_tensor_tensor(
                                out=tmp[:, :, :], in0=tmp[:, :, :], scalar=NEG_SLOPE,
                                in1=mxe[:, :, :], op0=AluOpType.mult, op1=AluOpType.add)
                            ex = ew.tile([P, K, nheads], bf16, tag="ex")
                            nc.scalar.activation(ex[:, :, :], tmp[:, :, :], Exp)
                            rhs = ew.tile([P, K, Racc], bf16, tag="rhs")
                            nc.vector.tensor_tensor(
                                out=rhs[:, :, 0:Cm].rearrange(
                                    "p k (h c) -> p k h c", h=nheads),
                                in0=hxg[:, :, 0:Cm].rearrange(
                                    "p k (h c) -> p k h c", h=nheads),
                                in1=ex[:, :, :].to_broadcast([P, K, nheads, chead]),
                                op=AluOpType.mult)
                            nc.vector.tensor_copy(out=rhs[:, :, Cm:Racc],
                                                  in_=ex[:, :, :])
                            acc = ps_acc.tile([P, Racc], f32, tag="acc", space="PSUM")
                            for k in range(K):
                                nc.tensor.matmul(out=acc[:, :], lhsT=mt[:, k, :],
                                                 rhs=rhs[:, k, :],
                                                 start=(k == 0), stop=(k == K - 1))
                            flush(b, acc, rows, ew)

                # ----- layer 1 flush: normalize, elu, layer-2 dense, store
                def flush1(b, acc, rows, ew):
                    recip = ew.tile([P, heads], f32, tag="recip")
                    nc.vector.reciprocal(recip[:rows, :], acc[:rows, HC : HC + heads])
                    h1 = ew.tile([P, HC], f32, tag="h1")
                    nc.vector.tensor_tensor(
                        out=h1[:rows, :].rearrange("p (h c) -> p h c", h=heads),
                        in0=acc[:rows, 0:HC].rearrange("p (h c) -> p h c", h=heads),
                        in1=recip[:rows, :].to_broadcast([rows, heads, hid]),
                        op=AluOpType.mult)
                    nc.vector.tensor_add(out=h1[:rows, :], in0=h1[:rows, :],
                                         in1=b1_sb[:rows, :])
                    mn = ew.tile([P, HC], f32, tag="mn")
                    nc.vector.tensor_scalar_min(mn[:rows, :], h1[:rows, :], 0.0)
                    nc.scalar.activation(mn[:rows, :], mn[:rows, :], Exp)
                    mx = ew.tile([P, HC], f32, tag="mx")
                    nc.vector.tensor_scalar_max(mx[:rows, :], h1[:rows, :], 0.0)
                    h1e = ew.tile([P, HC], bf16, tag="h1e")
                    nc.vector.scalar_tensor_tensor(
                        out=h1e[:rows, :], in0=mn[:rows, :], scalar=-1.0,
                        in1=mx[:rows, :], op0=AluOpType.add, op1=AluOpType.add)
                    tp = ew.tile([P, 2, P], bf16, tag="tp")
                    h2p = ps_small.tile([P, W2C], f32, tag="h2p", space="PSUM")
                    for ch in range(2):
                        tps = ps_tr.tile([P, P], bf16, tag="trp", space="PSUM")
                        nc.tensor.transpose(out=tps[:], in_=h1e[:, ch * P:(ch + 1) * P],
                                            identity=ident_sb[:])
                        nc.scalar.copy(out=tp[:, ch, :], in_=tps[:])
                        nc.tensor.matmul(
                            out=h2p[:rows, :],
                            lhsT=tp[:, ch, 0:rows],
                            rhs=w2_sb[:, ch * W2C:(ch + 1) * W2C],
                            start=(ch == 0), stop=(ch == 1))
                    st2 = ew.tile([P, R2], bf16, tag="st2")
                    nc.vector.memset(st2[:rows, G2W:R2], 0.0)
                    nc.scalar.copy(out=st2[:rows, 0:G2W], in_=h2p[:rows, 0:G2W])
                    nc.sync.dma_start(out=hx2in[b * P : b * P + rows, :],
                                      in_=st2[:rows, :])
                    se2 = ew.tile([P, 1], bf16, tag="se2")
                    nc.scalar.copy(out=se2[:rows, :], in_=h2p[:rows, G2W:W2C])
                    nc.sync.dma_start(out=edt2[b * P : b * P + rows, :],
                                      in_=se2[:rows, :])

                edge_layer(1, hx, R1, G1W, edt, heads, hid, flush1)

                # ----- AllGather hx2 (explicitly fenced)
                tc.strict_bb_all_engine_barrier()
                nc.gpsimd.collective_compute(
                    "AllGather", AluOpType.bypass,
                    replica_groups=[list(range(n_cores))],
                    ins=[hx2in[:, :]], outs=[hx2[:, :]])
                tc.strict_bb_all_engine_barrier()

                # ----- layer-2 edge pass (gathers straight from Shared hx2)
                def flush2(b, acc, rows, ew):
                    recip = ew.tile([P, 1], f32, tag="recip2")
                    nc.vector.reciprocal(recip[:rows, :], acc[:rows, cout : cout + 1])
                    o = ew.tile([P, cout], f32, tag="o")
                    nc.vector.tensor_tensor(
                        out=o[:rows, :], in0=acc[:rows, 0:cout],
                        in1=recip[:rows, :].to_broadcast([rows, cout]),
                        op=AluOpType.mult)
                    nc.vector.tensor_add(out=o[:rows, :], in0=o[:rows, :],
                                         in1=b2_sb[:rows, :])
                    nc.sync.dma_start(out=out[b * P : b * P + rows, :],
                                      in_=o[:rows, :])

                edge_layer(2, hx2, R2, G2W, edt2, 1, cout, flush2)

    nc.compile()
    return nc


def host_prep(x, edge_index, W1, a_src1, a_dst1, b1, W2, a_src2, a_dst2, b2,
              n_cores):
    """Plan the edge partition and build per-core input maps."""
    x = np.asarray(x, np.float32)
    n_nodes, cin = x.shape
    heads, hid = np.asarray(a_src1).shape
    cout = np.asarray(W2).shape[1]
    npc = n_nodes // n_cores

    loops = np.arange(n_nodes, dtype=np.int64)
    src = np.concatenate([np.asarray(edge_index[0], np.int64), loops])
    dst = np.concatenate([np.asarray(edge_index[1], np.int64), loops])
    pl = plan(src, dst, n_nodes, n_cores)

    W1 = np.asarray(W1, np.float32)
    W1h = W1.reshape(cin, heads, hid)
    Wa1 = np.einsum("khc,hc->kh", W1h, np.asarray(a_src1, np.float32))
    Wb1 = np.einsum("khc,hc->kh", W1h, np.asarray(a_dst1, np.float32))
    Wx1 = np.concatenate([W1, Wa1], axis=1).astype(BF)

    W2 = np.asarray(W2, np.float32)
    Wa2 = (W2 * np.asarray(a_src2, np.float32)).sum(1, keepdims=True)
    Wb2 = (W2 * np.asarray(a_dst2, np.float32)).sum(1, keepdims=True)
    W2e = np.concatenate([W2, Wa2, Wb2], axis=1)
    Wx2 = np.ascontiguousarray(
        np.concatenate([W2e[:P], W2e[P:]], axis=1)).astype(BF)

    xTb = np.ascontiguousarray(x.T).astype(BF)
    common = {
        "xT": xTb,
        "Wx1": Wx1,
        "Wb1": np.ascontiguousarray(Wb1).astype(BF),
        "Wx2": Wx2,
        "b1r": np.tile(np.asarray(b1, np.float32)[None, :], (P, 1)),
        "b2r": np.tile(np.asarray(b2, np.float32)[None, :], (P, 1)),
        "identb": np.eye(P, dtype=np.float32).astype(BF),
    }
    in_maps = []
    for c in range(n_cores):
        m = dict(common)
        m["xTl"] = np.ascontiguousarray(xTb[:, c * npc:(c + 1) * npc])
        m["srcw"] = np.ascontiguousarray(pl["srcw"][c])
        m["mtb"] = np.ascontiguousarray(pl["mtb"][c])
        m["mtTb"] = np.ascontiguousarray(pl["mtTb"][c])
        in_maps.append(m)
    return pl, (n_nodes, cin, heads, hid, cout), in_maps


def run_gat(x, edge_index, W1, a_src1, a_dst1, b1, W2, a_src2, a_dst2, b2,
            n_cores=8, trace=False):
    pl, (n_nodes, cin, heads, hid, cout), in_maps = host_prep(
        x, edge_index, W1, a_src1, a_dst1, b1, W2, a_src2, a_dst2, b2, n_cores)
    nc = build(pl, n_nodes, cin, heads, hid, cout, n_cores)
    res = bass_utils.run_bass_kernel_spmd(
        nc, in_maps, core_ids=list(range(n_cores)), trace=trace)
    outp = np.concatenate([res.results[c]["out"] for c in range(n_cores)], axis=0)
    return outp[:n_nodes], res


def kernel(**inputs):
    """Full-input GAT kernel: shards internally across 8 NeuronCores."""
    x = np.asarray(inputs["x"], np.float32)
    edge_index = np.asarray(inputs["edge_index"])
    outp, _ = run_gat(
        x, edge_index,
        inputs["W1"], inputs["a_src1"], inputs["a_dst1"], inputs["b1"],
        inputs["W2"], inputs["a_src2"], inputs["a_dst2"], inputs["b2"],
        n_cores=8, trace=bool(int(os.environ.get("GAT_TRACE", "0"))))
    return outp.astype(np.float32)


# revision 14
# speedup vs baseline: 1.6939x; 1.0091x over previous
"""GAT (2-layer, PyG-style) on 8 Trainium2 NeuronCores via Bass/Tile.

v4: dst-sharded nodes+edges across 8 cores.
 - dma_gather instructions round-robin over 4 SWDGE queues (4 Q7 core
   pairs emit descriptors concurrently; 3.9x emission throughput).
 - e_dst per edge via one-hot matmuls: host-precomputed fp8 one-hot
   matrices (mtT for dst->edge broadcast, mt for edge->dst segment sum)
   stream from DRAM; fp8 lhsT x bf16 rhs matmuls are exact for 0/1
   weights.  No PE transposes, no vector is_eq.
 - phase 1 stores hx rows as full contiguous 768B rows (few large DMA
   descriptors instead of 128 strided ones per block).
 - layer-2 table is gathered directly from the AllGather Shared-space
   output (no Shared->Local bounce copy).
The same index/one-hot arrays drive both layers (identical edge plan).
"""
import os
import sys

sys.path.insert(0, "/opt/trn_rl_repo")

import numpy as np
import ml_dtypes

import concourse.bass as bass
import concourse.mybir as mybir
import concourse.tile as tile
from concourse import bacc, bass_utils, library_config
from concourse.alu_op_type import AluOpType

P = 128
NEG_SLOPE = 0.2
GMAX = 8            # max idx columns per dma_gather = 1024 idx (HW limit)
BF = ml_dtypes.bfloat16
F8 = ml_dtypes.float8_e4m3
SPLIT = 32768       # int16 idx limit for dma_gather


def wrap_idx(vals):
    """idx sequence (len%128==0, 0<=v<32768) -> wrapped [128, len//16] int16."""
    a = np.asarray(vals, np.int64)
    assert len(a) % 128 == 0 and a.min() >= 0 and a.max() < SPLIT
    w = a.reshape(-1, 16).T.astype(np.int16)
    return np.tile(w, (8, 1))


def plan(src, dst, n_nodes, n_cores):
    npc = n_nodes // n_cores
    nblk = (npc + P - 1) // P
    order = np.argsort(dst, kind="stable")
    src_s = src[order].astype(np.int64)
    dst_s = dst[order].astype(np.int64)

    per = []            # [core][block] = (src_lo, src_hi, dst_lo_lo, dst_lo_hi)
    KL = np.zeros(nblk, np.int64)
    KH = np.zeros(nblk, np.int64)
    for c in range(n_cores):
        base = c * npc
        rows = []
        for b in range(nblk):
            n0 = base + b * P
            n1 = base + min((b + 1) * P, npc)
            e0 = np.searchsorted(dst_s, n0, side="left")
            e1 = np.searchsorted(dst_s, n1, side="left")
            s = src_s[e0:e1]
            dl = dst_s[e0:e1] - base          # core-local dst
            m = s < SPLIT
            rows.append((s[m], s[~m] - SPLIT, dl[m], dl[~m]))
            KL[b] = max(KL[b], (m.sum() + P - 1) // P)
            KH[b] = max(KH[b], ((~m).sum() + P - 1) // P)
        per.append(rows)

    Kb = KL + KH
    tob = np.concatenate([[0], np.cumsum(Kb)]).astype(np.int64)
    Ttot = int(Kb.sum())
    srcw = np.zeros((n_cores, P, Ttot * 8), np.int16)
    slot = np.full((n_cores, P, Ttot), -1, np.int64)
    for c in range(n_cores):
        for b in range(nblk):
            slo, shi, dlo, dhi = per[c][b]
            kl, kh = int(KL[b]), int(KH[b])
            t0 = int(tob[b])
            n0b = b * P
            for ss, dd, K, toff in [(slo, dlo, kl, t0), (shi, dhi, kh, t0 + kl)]:
                n = len(ss)
                npad = K * P
                if npad == 0:
                    continue
                a = np.zeros(npad, np.int64)
                a[:n] = ss
                sl = np.full(npad, -1, np.int64)
                sl[:n] = (dd - n0b)
                srcw[c, :, toff * 8:(toff + K) * 8] = wrap_idx(a)
                slot[c, :, toff:toff + K] = sl.reshape(K, P).T
    # one-hot matrices, fp8: mt[p, t*128+q] = (slot[p,t]==q)  (lhsT for
    # the segment-sum edge->slot matmul); mtT[q, t*128+p] = same
    # transposed (lhsT for the dst->edge e_dst broadcast matmul).
    mtb = np.zeros((n_cores, P, Ttot * P), np.uint8)
    mtTb = np.zeros((n_cores, P, Ttot * P), np.uint8)
    one = np.float32(1.0).astype(F8).view(np.uint8)
    for c in range(n_cores):
        pp, tt = np.nonzero(slot[c] >= 0)
        qq = slot[c][pp, tt]
        mtb[c, pp, tt * P + qq] = one
        mtTb[c, qq, tt * P + pp] = one
    return dict(npc=npc, nblk=nblk, KL=KL, KH=KH, Kb=Kb, tob=tob, Ttot=Ttot,
                Kmax=int(Kb.max()), srcw=srcw,
                mtb=mtb.view(F8), mtTb=mtTb.view(F8))


def raw_dma_gather(nc, out_ap, in_ap, idxs_ap, num_idxs, elem_size, elem_step,
                   queue_num):
    """dma_gather (non-transpose, DRAM source) without the %256 payload
    restriction; row stride (elem_step elements) must be a 256B multiple."""
    g = nc.gpsimd
    stride_bytes = elem_step * mybir.dt.size(in_ap.dtype)
    sb256 = stride_bytes // 256
    assert stride_bytes % 256 == 0 and sb256 < 256
    _in_ap = g.lower_ap_dma(in_ap, for_custom_bir_dma=True)
    _idxs_ap = g.lower_ap(idxs_ap)
    _out_ap = g.lower_ap(out_ap)
    return g.add_instruction(
        mybir.InstDMAGatherAnt(
            name=g.bass.get_next_instruction_name(),
            ins=[*_in_ap, _idxs_ap, g.lower_val_access(g.to_reg(num_idxs))],
            outs=[_out_ap], transpose=False, num_idxs=num_idxs,
            elem_size=elem_size, stride_bytes_256=sb256, gen_mode=0,
            single_packet=True, queue_num=queue_num, sbuf_tokens_per_rank=0,
            sbuf_free_dim_per_rank=0, sbuf_free_dim_pad_per_rank=0,
            sbuf_byte_offset=0))


def build(pl, n_nodes, cin, heads, hid, cout, n_cores):
    HC = heads * hid            # 256
    G1W = HC + heads            # 264 = [h | es] gather payload
    R1 = 384                    # hx row stride (768B)
    G2W = cout + 2              # 66  = [h2 | 1.0 | es2] gather payload
    R2 = 128                    # hx2 row stride (256B)
    npc, nblk = pl["npc"], pl["nblk"]
    KL, KH, Kb, tob = pl["KL"], pl["KH"], pl["Kb"], pl["tob"]
    Ttot, Kmax = pl["Ttot"], pl["Kmax"]
    NT1 = (n_nodes + P - 1) // P

    nc = bacc.Bacc("TRN2", num_swdge_queues=4)
    f32 = mybir.dt.float32
    bf16 = mybir.dt.bfloat16
    fp8 = mybir.dt.float8e4
    i16 = mybir.dt.int16
    Exp = mybir.ActivationFunctionType.Exp

    xT = nc.dram_tensor("xT", [cin, n_nodes], bf16, kind="ExternalInput")
    xTl = nc.dram_tensor("xTl", [cin, npc], bf16, kind="ExternalInput")
    Wx1 = nc.dram_tensor("Wx1", [cin, G1W], bf16, kind="ExternalInput")
    Wb1 = nc.dram_tensor("Wb1", [cin, heads], bf16, kind="ExternalInput")
    Wx2 = nc.dram_tensor("Wx2", [P, 2 * (cout + 2)], bf16, kind="ExternalInput")
    b1r = nc.dram_tensor("b1r", [P, HC], f32, kind="ExternalInput")
    b2r = nc.dram_tensor("b2r", [P, cout], f32, kind="ExternalInput")
    identb = nc.dram_tensor("identb", [P, P], bf16, kind="ExternalInput")
    srcw = nc.dram_tensor("srcw", [P, Ttot * 8], i16, kind="ExternalInput")
    mtb = nc.dram_tensor("mtb", [P, Ttot * P], fp8, kind="ExternalInput")
    mtTb = nc.dram_tensor("mtTb", [P, Ttot * P], fp8, kind="ExternalInput")
    out = nc.dram_tensor("out", [npc, cout], f32, kind="ExternalOutput")

    hx = nc.dram_tensor("hx", [n_nodes, R1], bf16)
    edt = nc.dram_tensor("edt", [npc, heads], bf16)
    edt2 = nc.dram_tensor("edt2", [npc, 1], bf16)
    hx2in = nc.dram_tensor("hx2in", [npc, R2], bf16)
    hx2 = nc.dram_tensor("hx2", [n_nodes, R2], bf16, addr_space="Shared")

    W2C = cout + 2              # 66 = [h2 | es2 | ed2] from the flush matmul
    Prelu = mybir.ActivationFunctionType.Prelu

    qctr = [0]

    def next_q():
        q = qctr[0] & 3
        qctr[0] += 1
        return q

    with tile.TileContext(nc) as tc:
        with tc.tile_pool(name="const", bufs=1) as cp:
            nc.gpsimd.load_library(library_config.mlp)
            ident_sb = cp.tile([P, P], bf16)
            b1_sb = cp.tile([P, HC], f32)
            b2_sb = cp.tile([P, cout], f32)
            w1_sb = cp.tile([cin, G1W], bf16)
            wb1_sb = cp.tile([cin, heads], bf16)
            w2_sb = cp.tile([P, 2 * W2C], bf16)
            sidx_sb = cp.tile([P, Ttot * 8], i16)
            nc.sync.dma_start(out=ident_sb[:], in_=identb[:, :])
            nc.sync.dma_start(out=b1_sb[:], in_=b1r[:, :])
            nc.sync.dma_start(out=b2_sb[:], in_=b2r[:, :])
            nc.sync.dma_start(out=w1_sb[:], in_=Wx1[:, :])
            nc.sync.dma_start(out=wb1_sb[:], in_=Wb1[:, :])
            nc.sync.dma_start(out=w2_sb[:], in_=Wx2[:, :])
            nc.sync.dma_start(out=sidx_sb[:], in_=srcw[:, :])

            # ---------- phase 1: hx = [x@W1 | es] bf16 for ALL nodes
            # (contiguous full-row stores); edt = x_local@Wb1 for LOCAL nodes
            XB = 8
            with tc.tile_pool(name="p1", bufs=3) as p1, \
                 tc.tile_pool(name="p1ps", bufs=4, space="PSUM") as p1ps:
                for j0 in range(0, NT1, XB):
                    jn = min(XB, NT1 - j0)
                    w = min(XB * P, n_nodes - j0 * P)
                    xt_sb = p1.tile([cin, XB * P], bf16, tag="xt")
                    nc.sync.dma_start(out=xt_sb[:, :w], in_=xT[:, j0 * P : j0 * P + w])
                    st = p1.tile([P, XB, R1], bf16, tag="st")
                    for k in range(jn):
                        rows = min(P, n_nodes - (j0 + k) * P)
                        ps = p1ps.tile([P, G1W], f32, tag="ps")
                        nc.tensor.matmul(out=ps[:rows, :],
                                         lhsT=xt_sb[:, k * P : k * P + rows],
                                         rhs=w1_sb[:, :], start=True, stop=True)
                        if k % 2 == 0:
                            nc.scalar.copy(out=st[:rows, k, 0:G1W], in_=ps[:rows, :])
                        else:
                            nc.vector.tensor_copy(out=st[:rows, k, 0:G1W], in_=ps[:rows, :])
                    full = jn if w == jn * P else jn - 1
                    if full:
                        nc.sync.dma_start(
                            out=hx[j0 * P : (j0 + full) * P, :].rearrange(
                                "(a b) c -> b a c", b=P),
                            in_=st[:, 0:full, :])
                    if full < jn:
                        rows = w - full * P
                        nc.sync.dma_start(
                            out=hx[(j0 + full) * P : j0 * P + w, 0:G1W],
                            in_=st[:rows, full, 0:G1W])
                for j0 in range(0, nblk, XB):
                    jn = min(XB, nblk - j0)
                    w = min(XB * P, npc - j0 * P)
                    xt_sb = p1.tile([cin, XB * P], bf16, tag="xt")
                    nc.sync.dma_start(out=xt_sb[:, :w], in_=xTl[:, j0 * P : j0 * P + w])
                    ste = p1.tile([P, XB, heads], bf16, tag="ste")
                    for k in range(jn):
                        rows = min(P, npc - (j0 + k) * P)
                        pse = p1ps.tile([P, heads], f32, tag="pse")
                        nc.tensor.matmul(out=pse[:rows, :],
                                         lhsT=xt_sb[:, k * P : k * P + rows],
                                         rhs=wb1_sb[:, :], start=True, stop=True)
                        if k % 2 == 0:
                            nc.scalar.copy(out=ste[:rows, k, :], in_=pse[:rows, :])
                        else:
                            nc.vector.tensor_copy(out=ste[:rows, k, :], in_=pse[:rows, :])
                    full = jn if w == jn * P else jn - 1
                    if full:
                        nc.sync.dma_start(
                            out=edt[j0 * P : (j0 + full) * P, :].rearrange(
                                "(a b) c -> b a c", b=P),
                            in_=ste[:, 0:full, :])
                    if full < jn:
                        rows = w - full * P
                        nc.sync.dma_start(
                            out=edt[(j0 + full) * P : j0 * P + w, :],
                            in_=ste[:rows, full, :])

            tc.strict_bb_all_engine_barrier()

            with tc.tile_pool(name="ps_acc", bufs=2, space="PSUM") as ps_acc, \
                 tc.tile_pool(name="ps_tr", bufs=2, space="PSUM") as ps_tr, \
                 tc.tile_pool(name="ps_small", bufs=2, space="PSUM") as ps_small, \
                 tc.tile_pool(name="ps_edp", bufs=2, space="PSUM") as ps_edp:

                def edge_layer(layer, table, RowW, gwidth, edtab, nheads,
                               chead, es_off, flush):
                    Cm = nheads * chead
                    Racc = Cm + nheads
                    lo_end = min(SPLIT, n_nodes)
                    with tc.tile_pool(name=f"eg{layer}", bufs=4) as eg, \
                         tc.tile_pool(name=f"ew{layer}", bufs=3) as ew, \
                         tc.tile_pool(name=f"em{layer}", bufs=3) as em:
                        for b in range(nblk):
                            kl, kh = int(KL[b]), int(KH[b])
                            K = kl + kh
                            t0 = int(tob[b])
                            rows = min(P, npc - b * P)
                            # per-block e_dst rows: one contiguous DMA
                            edr = ew.tile([P, nheads], bf16, tag="edr")
                            if rows < P:
                                nc.vector.memset(edr[:, :], 0.0)
                            nc.sync.dma_start(
                                out=edr[:rows, :],
                                in_=edtab[b * P : b * P + rows, :])
                            # one-hot pair for this block (fp8, flat 2D loads)
                            mt = em.tile([P, Kmax * P], fp8, tag="mt")
                            nc.sync.dma_start(
                                out=mt[:, 0:K * P],
                                in_=mtb[:, t0 * P:(t0 + K) * P])
                            mtT = em.tile([P, Kmax * P], fp8, tag="mtT")
                            nc.sync.dma_start(
                                out=mtT[:, 0:K * P],
                                in_=mtTb[:, t0 * P:(t0 + K) * P])
                            hxg = eg.tile([P, K, gwidth], bf16, tag="hxg")
                            for c0 in range(0, kl, GMAX):
                                ncc = min(GMAX, kl - c0)
                                raw_dma_gather(
                                    nc, hxg[:, c0:c0 + ncc, :],
                                    table[0:lo_end, 0:gwidth],
                                    sidx_sb[:, (t0 + c0) * 8:(t0 + c0 + ncc) * 8],
                                    ncc * P, gwidth, RowW, next_q())
                            for c0 in range(kl, K, GMAX):
                                ncc = min(GMAX, K - c0)
                                raw_dma_gather(
                                    nc, hxg[:, c0:c0 + ncc, :],
                                    table[SPLIT:n_nodes, 0:gwidth],
                                    sidx_sb[:, (t0 + c0) * 8:(t0 + c0 + ncc) * 8],
                                    ncc * P, gwidth, RowW, next_q())
                            # e_dst per edge: edp[p, k*nh:] = edr[slot[p,k], :]
                            edp = ps_edp.tile([P, Kmax * nheads], f32,
                                              tag="edp", space="PSUM")
                            for k in range(K):
                                nc.tensor.matmul(
                                    out=edp[:, k * nheads:(k + 1) * nheads],
                                    lhsT=mtT[:, k * P:(k + 1) * P], rhs=edr[:, :],
                                    start=True, stop=True)
                            tmp = ew.tile([P, K, nheads], f32, tag="tmp")
                            nc.vector.tensor_tensor(
                                out=tmp[:, :, :],
                                in0=hxg[:, :, es_off : es_off + nheads],
                                in1=edp[:, 0:K * nheads].rearrange(
                                    "p (k h) -> p k h", k=K), op=AluOpType.add)
                            # exp(leaky_relu(.)) on the scalar engine
                            nc.scalar.activation(tmp[:, :, :], tmp[:, :, :],
                                                 Prelu, alpha=NEG_SLOPE)
                            ex = ew.tile([P, K, nheads], bf16, tag="ex")
                            nc.scalar.activation(ex[:, :, :], tmp[:, :, :], Exp)
                            rhs = ew.tile([P, K, Racc], bf16, tag="rhs")
                            if nheads == 1:
                                # table row is [h | 1.0 | es]: one multiply
                                # yields [h*ex | ex] including the denominator
                                nc.vector.tensor_tensor(
                                    out=rhs[:, :, :],
                                    in0=hxg[:, :, 0:Racc],
                                    in1=ex[:, :, 0:1].to_broadcast([P, K, Racc]),
                                    op=AluOpType.mult)
                            else:
                                nc.vector.tensor_tensor(
                                    out=rhs[:, :, 0:Cm].rearrange(
                                        "p k (h c) -> p k h c", h=nheads),
                                    in0=hxg[:, :, 0:Cm].rearrange(
                                        "p k (h c) -> p k h c", h=nheads),
                                    in1=ex[:, :, :].to_broadcast(
                                        [P, K, nheads, chead]),
                                    op=AluOpType.mult)
                                nc.vector.tensor_copy(out=rhs[:, :, Cm:Racc],
                                                      in_=ex[:, :, :])
                            acc = ps_acc.tile([P, Racc], f32, tag="acc", space="PSUM")
                            for k in range(K):
                                nc.tensor.matmul(out=acc[:, :],
                                                 lhsT=mt[:, k * P:(k + 1) * P],
                                                 rhs=rhs[:, k, :],
                                                 start=(k == 0), stop=(k == K - 1))
                            flush(b, acc, rows, ew)

                # ----- layer 1 flush: normalize, elu, layer-2 dense, store
                def flush1(b, acc, rows, ew):
                    recip = ew.tile([P, heads], f32, tag="recip")
                    nc.vector.reciprocal(recip[:rows, :], acc[:rows, HC : HC + heads])
                    h1 = ew.tile([P, HC], f32, tag="h1")
                    nc.vector.tensor_tensor(
                        out=h1[:rows, :].rearrange("p (h c) -> p h c", h=heads),
                        in0=acc[:rows, 0:HC].rearrange("p (h c) -> p h c", h=heads),
                        in1=recip[:rows, :].to_broadcast([rows, heads, hid]),
                        op=AluOpType.mult)
                    nc.vector.tensor_add(out=h1[:rows, :], in0=h1[:rows, :],
                                         in1=b1_sb[:rows, :])
                    mn = ew.tile([P, HC], f32, tag="mn")
                    nc.vector.tensor_scalar_min(mn[:rows, :], h1[:rows, :], 0.0)
                    nc.scalar.activation(mn[:rows, :], mn[:rows, :], Exp)
                    mx = ew.tile([P, HC], f32, tag="mx")
                    nc.vector.tensor_scalar_max(mx[:rows, :], h1[:rows, :], 0.0)
                    h1e = ew.tile([P, HC], bf16, tag="h1e")
                    nc.vector.scalar_tensor_tensor(
                        out=h1e[:rows, :], in0=mn[:rows, :], scalar=-1.0,
                        in1=mx[:rows, :], op0=AluOpType.add, op1=AluOpType.add)
                    tp = ew.tile([P, 2, P], bf16, tag="tp")
                    h2p = ps_small.tile([P, W2C], f32, tag="h2p", space="PSUM")
                    for ch in range(2):
                        tps = ps_tr.tile([P, P], bf16, tag="trp", space="PSUM")
                        nc.tensor.transpose(out=tps[:], in_=h1e[:, ch * P:(ch + 1) * P],
                                            identity=ident_sb[:])
                        nc.scalar.copy(out=tp[:, ch, :], in_=tps[:])
                        nc.tensor.matmul(
                            out=h2p[:rows, :],
                            lhsT=tp[:, ch, 0:rows],
                            rhs=w2_sb[:, ch * W2C:(ch + 1) * W2C],
                            start=(ch == 0), stop=(ch == 1))
                    st2 = ew.tile([P, R2], bf16, tag="st2")
                    nc.vector.memset(st2[:rows, G2W:R2], 0.0)
                    nc.vector.memset(st2[:rows, cout:cout + 1], 1.0)
                    nc.scalar.copy(out=st2[:rows, 0:cout], in_=h2p[:rows, 0:cout])
                    nc.scalar.copy(out=st2[:rows, cout + 1:cout + 2],
                                   in_=h2p[:rows, cout:cout + 1])
                    nc.sync.dma_start(out=hx2in[b * P : b * P + rows, :],
                                      in_=st2[:rows, :])
                    se2 = ew.tile([P, 1], bf16, tag="se2")
                    nc.scalar.copy(out=se2[:rows, :],
                                   in_=h2p[:rows, cout + 1:cout + 2])
                    nc.sync.dma_start(out=edt2[b * P : b * P + rows, :],
                                      in_=se2[:rows, :])

                edge_layer(1, hx, R1, G1W, edt, heads, hid, HC, flush1)

                # ----- AllGather hx2 (issued without a pre-barrier; Tile
                # orders it after the hx2in stores via data deps)
                nc.gpsimd.collective_compute(
                    "AllGather", AluOpType.bypass,
                    replica_groups=[list(range(n_cores))],
                    ins=[hx2in[:, :]], outs=[hx2[:, :]])
                tc.strict_bb_all_engine_barrier()

                # ----- layer-2 edge pass (gathers straight from Shared hx2)
                def flush2(b, acc, rows, ew):
                    recip = ew.tile([P, 1], f32, tag="recip2")
                    nc.vector.reciprocal(recip[:rows, :], acc[:rows, cout : cout + 1])
                    o = ew.tile([P, cout], f32, tag="o")
                    nc.vector.tensor_tensor(
                        out=o[:rows, :], in0=acc[:rows, 0:cout],
                        in1=recip[:rows, :].to_broadcast([rows, cout]),
                        op=AluOpType.mult)
                    nc.vector.tensor_add(out=o[:rows, :], in0=o[:rows, :],
                                         in1=b2_sb[:rows, :])
                    nc.sync.dma_start(out=out[b * P : b * P + rows, :],
                                      in_=o[:rows, :])

                edge_layer(2, hx2, R2, G2W, edt2, 1, cout, cout + 1, flush2)

    nc.compile()
    return nc


def host_prep(x, edge_index, W1, a_src1, a_dst1, b1, W2, a_src2, a_dst2, b2,
              n_cores):
    """Plan the edge partition and build per-core input maps."""
    x = np.asarray(x, np.float32)
    n_nodes, cin = x.shape
    heads, hid = np.asarray(a_src1).shape
    cout = np.asarray(W2).shape[1]
    npc = n_nodes // n_cores

    loops = np.arange(n_nodes, dtype=np.int64)
    src = np.concatenate([np.asarray(edge_index[0], np.int64), loops])
    dst = np.concatenate([np.asarray(edge_index[1], np.int64), loops])
    pl = plan(src, dst, n_nodes, n_cores)

    W1 = np.asarray(W1, np.float32)
    W1h = W1.reshape(cin, heads, hid)
    Wa1 = np.einsum("khc,hc->kh", W1h, np.asarray(a_src1, np.float32))
    Wb1 = np.einsum("khc,hc->kh", W1h, np.asarray(a_dst1, np.float32))
    Wx1 = np.concatenate([W1, Wa1], axis=1).astype(BF)

    W2 = np.asarray(W2, np.float32)
    Wa2 = (W2 * np.asarray(a_src2, np.float32)).sum(1, keepdims=True)
    Wb2 = (W2 * np.asarray(a_dst2, np.float32)).sum(1, keepdims=True)
    W2e = np.concatenate([W2, Wa2, Wb2], axis=1)
    Wx2 = np.ascontiguousarray(
        np.concatenate([W2e[:P], W2e[P:]], axis=1)).astype(BF)

    xTb = np.ascontiguousarray(x.T).astype(BF)
    common = {
        "xT": xTb,
        "Wx1": Wx1,
        "Wb1": np.ascontiguousarray(Wb1).astype(BF),
        "Wx2": Wx2,
        "b1r": np.tile(np.asarray(b1, np.float32)[None, :], (P, 1)),
        "b2r": np.tile(np.asarray(b2, np.float32)[None, :], (P, 1)),
        "identb": np.eye(P, dtype=np.float32).astype(BF),
    }
    in_maps = []
    for c in range(n_cores):
        m = dict(common)
        m["xTl"] = np.ascontiguousarray(xTb[:, c * npc:(c + 1) * npc])
        m["srcw"] = np.ascontiguousarray(pl["srcw"][c])
        m["mtb"] = np.ascontiguousarray(pl["mtb"][c])
        m["mtTb"] = np.ascontiguousarray(pl["mtTb"][c])
        in_maps.append(m)
    return pl, (n_nodes, cin, heads, hid, cout), in_maps


def run_gat(x, edge_index, W1, a_src1, a_dst1, b1, W2, a_src2, a_dst2, b2,
            n_cores=8, trace=False):
    pl, (n_nodes, cin, heads, hid, cout), in_maps = host_prep(
        x, edge_index, W1, a_src1, a_dst1, b1, W2, a_src2, a_dst2, b2, n_cores)
    nc = build(pl, n_nodes, cin, heads, hid, cout, n_cores)
    res = bass_utils.run_bass_kernel_spmd(
        nc, in_maps, core_ids=list(range(n_cores)), trace=trace)
    outp = np.concatenate([res.results[c]["out"] for c in range(n_cores)], axis=0)
    return outp[:n_nodes], res


def kernel(**inputs):
    """Full-input GAT kernel: shards internally across 8 NeuronCores."""
    x = np.asarray(inputs["x"], np.float32)
    edge_index = np.asarray(inputs["edge_index"])
    outp, _ = run_gat(
        x, edge_index,
        inputs["W1"], inputs["a_src1"], inputs["a_dst1"], inputs["b1"],
        inputs["W2"], inputs["a_src2"], inputs["a_dst2"], inputs["b2"],
        n_cores=8, trace=bool(int(os.environ.get("GAT_TRACE", "0"))))
    return outp.astype(np.float32)


# revision 19
# speedup vs baseline: 1.7565x; 1.0370x over previous
"""GAT (2-layer, PyG-style) on 8 Trainium2 NeuronCores via Bass/Tile.

v4: dst-sharded nodes+edges across 8 cores.
 - dma_gather instructions round-robin over 4 SWDGE queues (4 Q7 core
   pairs emit descriptors concurrently; 3.9x emission throughput).
 - e_dst per edge via one-hot matmuls: host-precomputed fp8 one-hot
   matrices (mtT for dst->edge broadcast, mt for edge->dst segment sum)
   stream from DRAM; fp8 lhsT x bf16 rhs matmuls are exact for 0/1
   weights.  No PE transposes, no vector is_eq.
 - phase 1 stores hx rows as full contiguous 768B rows (few large DMA
   descriptors instead of 128 strided ones per block).
 - layer-2 table is gathered directly from the AllGather Shared-space
   output (no Shared->Local bounce copy).
The same index/one-hot arrays drive both layers (identical edge plan).
"""
import os
import sys

sys.path.insert(0, "/opt/trn_rl_repo")

import numpy as np
import ml_dtypes

import concourse.bass as bass
import concourse.mybir as mybir
import concourse.tile as tile
from concourse import bacc, bass_utils, library_config
from concourse.alu_op_type import AluOpType

P = 128
NEG_SLOPE = 0.2
GMAX = 8            # max idx columns per dma_gather = 1024 idx (HW limit)
BF = ml_dtypes.bfloat16
F8 = ml_dtypes.float8_e4m3
SPLIT = 32768       # int16 idx limit for dma_gather


def wrap_idx(vals):
    """idx sequence (len%128==0, 0<=v<32768) -> wrapped [128, len//16] int16."""
    a = np.asarray(vals, np.int64)
    assert len(a) % 128 == 0 and a.min() >= 0 and a.max() < SPLIT
    w = a.reshape(-1, 16).T.astype(np.int16)
    return np.tile(w, (8, 1))


def plan(src, dst, n_nodes, n_cores):
    npc = n_nodes // n_cores
    nblk = (npc + P - 1) // P
    order = np.argsort(dst, kind="stable")
    src_s = src[order].astype(np.int64)
    dst_s = dst[order].astype(np.int64)

    per = []            # [core][block] = (src_lo, src_hi, dst_lo_lo, dst_lo_hi)
    KL = np.zeros(nblk, np.int64)
    KH = np.zeros(nblk, np.int64)
    for c in range(n_cores):
        base = c * npc
        rows = []
        for b in range(nblk):
            n0 = base + b * P
            n1 = base + min((b + 1) * P, npc)
            e0 = np.searchsorted(dst_s, n0, side="left")
            e1 = np.searchsorted(dst_s, n1, side="left")
            s = src_s[e0:e1]
            dl = dst_s[e0:e1] - base          # core-local dst
            m = s < SPLIT
            rows.append((s[m], s[~m] - SPLIT, dl[m], dl[~m]))
            KL[b] = max(KL[b], (m.sum() + P - 1) // P)
            KH[b] = max(KH[b], ((~m).sum() + P - 1) // P)
        per.append(rows)

    Kb = KL + KH
    tob = np.concatenate([[0], np.cumsum(Kb)]).astype(np.int64)
    Ttot = int(Kb.sum())
    srcw = np.zeros((n_cores, P, Ttot * 8), np.int16)
    slot = np.full((n_cores, P, Ttot), -1, np.int64)
    for c in range(n_cores):
        for b in range(nblk):
            slo, shi, dlo, dhi = per[c][b]
            kl, kh = int(KL[b]), int(KH[b])
            t0 = int(tob[b])
            n0b = b * P
            for ss, dd, K, toff in [(slo, dlo, kl, t0), (shi, dhi, kh, t0 + kl)]:
                n = len(ss)
                npad = K * P
                if npad == 0:
                    continue
                a = np.zeros(npad, np.int64)
                a[:n] = ss
                sl = np.full(npad, -1, np.int64)
                sl[:n] = (dd - n0b)
                srcw[c, :, toff * 8:(toff + K) * 8] = wrap_idx(a)
                slot[c, :, toff:toff + K] = sl.reshape(K, P).T
    # one-hot matrices, fp8: mt[p, t*128+q] = (slot[p,t]==q)  (lhsT for
    # the segment-sum edge->slot matmul); mtT[q, t*128+p] = same
    # transposed (lhsT for the dst->edge e_dst broadcast matmul).
    # Packed per BLOCK as one contiguous [P, K*P (mt) | K*P (mtT)] chunk so
    # the per-block stream is a single fully-sequential DRAM read.
    mtb = np.zeros((n_cores, P, Ttot * P), np.uint8)
    mtTb = np.zeros((n_cores, P, Ttot * P), np.uint8)
    one = np.float32(1.0).astype(F8).view(np.uint8)
    for c in range(n_cores):
        pp, tt = np.nonzero(slot[c] >= 0)
        qq = slot[c][pp, tt]
        mtb[c, pp, tt * P + qq] = one
        mtTb[c, qq, tt * P + pp] = one
    mtc = np.zeros((n_cores, 2 * Ttot * P * P), np.uint8)
    for c in range(n_cores):
        off = 0
        for b in range(nblk):
            t0, t1 = int(tob[b]), int(tob[b + 1])
            w = (t1 - t0) * P
            blk = np.concatenate(
                [mtb[c][:, t0 * P:t1 * P], mtTb[c][:, t0 * P:t1 * P]], axis=1)
            mtc[c, off:off + 2 * w * P] = blk.reshape(-1)
            off += 2 * w * P
    return dict(npc=npc, nblk=nblk, KL=KL, KH=KH, Kb=Kb, tob=tob, Ttot=Ttot,
                Kmax=int(Kb.max()), srcw=srcw, mtc=mtc.view(F8))


def raw_dma_gather(nc, out_ap, in_ap, idxs_ap, num_idxs, elem_size, elem_step,
                   queue_num):
    """dma_gather (non-transpose, DRAM source) without the %256 payload
    restriction; row stride (elem_step elements) must be a 256B multiple."""
    g = nc.gpsimd
    stride_bytes = elem_step * mybir.dt.size(in_ap.dtype)
    sb256 = stride_bytes // 256
    assert stride_bytes % 256 == 0 and sb256 < 256
    _in_ap = g.lower_ap_dma(in_ap, for_custom_bir_dma=True)
    _idxs_ap = g.lower_ap(idxs_ap)
    _out_ap = g.lower_ap(out_ap)
    return g.add_instruction(
        mybir.InstDMAGatherAnt(
            name=g.bass.get_next_instruction_name(),
            ins=[*_in_ap, _idxs_ap, g.lower_val_access(g.to_reg(num_idxs))],
            outs=[_out_ap], transpose=False, num_idxs=num_idxs,
            elem_size=elem_size, stride_bytes_256=sb256, gen_mode=0,
            single_packet=True, queue_num=queue_num, sbuf_tokens_per_rank=0,
            sbuf_free_dim_per_rank=0, sbuf_free_dim_pad_per_rank=0,
            sbuf_byte_offset=0))


def build(pl, n_nodes, cin, heads, hid, cout, n_cores):
    HC = heads * hid            # 256
    G1W = HC + heads            # 264 = [h | es] gather payload
    R1 = 384                    # hx row stride (768B)
    G2W = cout + 2              # 66  = [h2 | 1.0 | es2] gather payload
    R2 = 128                    # hx2 row stride (256B)
    npc, nblk = pl["npc"], pl["nblk"]
    KL, KH, Kb, tob = pl["KL"], pl["KH"], pl["Kb"], pl["tob"]
    Ttot, Kmax = pl["Ttot"], pl["Kmax"]
    NT1 = (n_nodes + P - 1) // P

    nc = bacc.Bacc("TRN2", num_swdge_queues=4)
    f32 = mybir.dt.float32
    bf16 = mybir.dt.bfloat16
    fp8 = mybir.dt.float8e4
    i16 = mybir.dt.int16
    Exp = mybir.ActivationFunctionType.Exp

    xT = nc.dram_tensor("xT", [cin, n_nodes], bf16, kind="ExternalInput")
    xTl = nc.dram_tensor("xTl", [cin, npc], bf16, kind="ExternalInput")
    Wx1 = nc.dram_tensor("Wx1", [cin, G1W], bf16, kind="ExternalInput")
    Wb1 = nc.dram_tensor("Wb1", [cin, heads], bf16, kind="ExternalInput")
    Wx2 = nc.dram_tensor("Wx2", [P, 2 * (cout + 2)], bf16, kind="ExternalInput")
    b1r = nc.dram_tensor("b1r", [P, HC], f32, kind="ExternalInput")
    b2r = nc.dram_tensor("b2r", [P, cout], f32, kind="ExternalInput")
    identb = nc.dram_tensor("identb", [P, P], bf16, kind="ExternalInput")
    srcw = nc.dram_tensor("srcw", [P, Ttot * 8], i16, kind="ExternalInput")
    mtc = nc.dram_tensor("mtc", [2 * Ttot * P * P], fp8, kind="ExternalInput")
    out = nc.dram_tensor("out", [npc, cout], f32, kind="ExternalOutput")

    hx = nc.dram_tensor("hx", [n_nodes, R1], bf16)
    edt = nc.dram_tensor("edt", [npc, heads], bf16)
    edt2 = nc.dram_tensor("edt2", [npc, 1], bf16)
    hx2in = nc.dram_tensor("hx2in", [npc, R2], bf16)
    hx2 = nc.dram_tensor("hx2", [n_nodes, R2], bf16, addr_space="Shared")

    W2C = cout + 2              # 66 = [h2 | es2 | ed2] from the flush matmul
    Prelu = mybir.ActivationFunctionType.Prelu

    qctr = [0]

    def next_q():
        q = qctr[0] & 3
        qctr[0] += 1
        return q

    with tile.TileContext(nc) as tc:
        with tc.tile_pool(name="const", bufs=1) as cp:
            nc.gpsimd.load_library(library_config.mlp)
            ident_sb = cp.tile([P, P], bf16)
            b1_sb = cp.tile([P, HC], f32)
            b2_sb = cp.tile([P, cout], f32)
            w1_sb = cp.tile([cin, G1W], bf16)
            wb1_sb = cp.tile([cin, heads], bf16)
            w2_sb = cp.tile([P, 2 * W2C], bf16)
            sidx_sb = cp.tile([P, Ttot * 8], i16)
            nc.sync.dma_start(out=ident_sb[:], in_=identb[:, :])
            nc.sync.dma_start(out=b1_sb[:], in_=b1r[:, :])
            nc.sync.dma_start(out=b2_sb[:], in_=b2r[:, :])
            nc.sync.dma_start(out=w1_sb[:], in_=Wx1[:, :])
            nc.sync.dma_start(out=wb1_sb[:], in_=Wb1[:, :])
            nc.sync.dma_start(out=w2_sb[:], in_=Wx2[:, :])
            nc.sync.dma_start(out=sidx_sb[:], in_=srcw[:, :])

            # ---------- phase 1: hx = [x@W1 | es] bf16 for ALL nodes
            # (contiguous full-row stores); edt = x_local@Wb1 for LOCAL nodes
            XB = 8
            with tc.tile_pool(name="p1", bufs=3) as p1, \
                 tc.tile_pool(name="p1ps", bufs=4, space="PSUM") as p1ps:
                for j0 in range(0, NT1, XB):
                    jn = min(XB, NT1 - j0)
                    w = min(XB * P, n_nodes - j0 * P)
                    xt_sb = p1.tile([cin, XB * P], bf16, tag="xt")
                    nc.sync.dma_start(out=xt_sb[:, :w], in_=xT[:, j0 * P : j0 * P + w])
                    st = p1.tile([P, XB, R1], bf16, tag="st")
                    for k in range(jn):
                        rows = min(P, n_nodes - (j0 + k) * P)
                        ps = p1ps.tile([P, G1W], f32, tag="ps")
                        nc.tensor.matmul(out=ps[:rows, :],
                                         lhsT=xt_sb[:, k * P : k * P + rows],
                                         rhs=w1_sb[:, :], start=True, stop=True)
                        if k % 2 == 0:
                            nc.scalar.copy(out=st[:rows, k, 0:G1W], in_=ps[:rows, :])
                        else:
                            nc.vector.tensor_copy(out=st[:rows, k, 0:G1W], in_=ps[:rows, :])
                    full = jn if w == jn * P else jn - 1
                    if full:
                        nc.sync.dma_start(
                            out=hx[j0 * P : (j0 + full) * P, :].rearrange(
                                "(a b) c -> b a c", b=P),
                            in_=st[:, 0:full, :])
                    if full < jn:
                        rows = w - full * P
                        nc.sync.dma_start(
                            out=hx[(j0 + full) * P : j0 * P + w, 0:G1W],
                            in_=st[:rows, full, 0:G1W])
                for j0 in range(0, nblk, XB):
                    jn = min(XB, nblk - j0)
                    w = min(XB * P, npc - j0 * P)
                    xt_sb = p1.tile([cin, XB * P], bf16, tag="xt")
                    nc.sync.dma_start(out=xt_sb[:, :w], in_=xTl[:, j0 * P : j0 * P + w])
                    ste = p1.tile([P, XB, heads], bf16, tag="ste")
                    for k in range(jn):
                        rows = min(P, npc - (j0 + k) * P)
                        pse = p1ps.tile([P, heads], f32, tag="pse")
                        nc.tensor.matmul(out=pse[:rows, :],
                                         lhsT=xt_sb[:, k * P : k * P + rows],
                                         rhs=wb1_sb[:, :], start=True, stop=True)
                        if k % 2 == 0:
                            nc.scalar.copy(out=ste[:rows, k, :], in_=pse[:rows, :])
                        else:
                            nc.vector.tensor_copy(out=ste[:rows, k, :], in_=pse[:rows, :])
                    full = jn if w == jn * P else jn - 1
                    if full:
                        nc.sync.dma_start(
                            out=edt[j0 * P : (j0 + full) * P, :].rearrange(
                                "(a b) c -> b a c", b=P),
                            in_=ste[:, 0:full, :])
                    if full < jn:
                        rows = w - full * P
                        nc.sync.dma_start(
                            out=edt[(j0 + full) * P : j0 * P + w, :],
                            in_=ste[:rows, full, :])

            tc.strict_bb_all_engine_barrier()

            with tc.tile_pool(name="ps_acc", bufs=2, space="PSUM") as ps_acc, \
                 tc.tile_pool(name="ps_tr", bufs=2, space="PSUM") as ps_tr, \
                 tc.tile_pool(name="ps_small", bufs=2, space="PSUM") as ps_small, \
                 tc.tile_pool(name="ps_edp", bufs=2, space="PSUM") as ps_edp:

                def edge_layer(layer, table, RowW, gwidth, edtab, nheads,
                               chead, es_off, flush):
                    Cm = nheads * chead
                    Racc = Cm + nheads
                    lo_end = min(SPLIT, n_nodes)
                    with tc.tile_pool(name=f"eg{layer}", bufs=4) as eg, \
                         tc.tile_pool(name=f"ew{layer}", bufs=3) as ew, \
                         tc.tile_pool(name=f"em{layer}", bufs=3) as em:
                        for b in range(nblk):
                            kl, kh = int(KL[b]), int(KH[b])
                            K = kl + kh
                            t0 = int(tob[b])
                            rows = min(P, npc - b * P)
                            # per-block e_dst rows: one contiguous DMA
                            edr = ew.tile([P, nheads], bf16, tag="edr")
                            if rows < P:
                                nc.vector.memset(edr[:, :], 0.0)
                            nc.sync.dma_start(
                                out=edr[:rows, :],
                                in_=edtab[b * P : b * P + rows, :])
                            # one-hot pair for this block: one sequential read
                            mtw = em.tile([P, 2 * Kmax * P], fp8, tag="mtw")
                            o0 = 2 * t0 * P * P
                            nc.sync.dma_start(
                                out=mtw[:, 0:2 * K * P],
                                in_=mtc[o0:o0 + 2 * K * P * P].rearrange(
                                    "(p c) -> p c", p=P))
                            mt = mtw[:, 0:K * P]
                            mtT = mtw[:, K * P:2 * K * P]
                            hxg = eg.tile([P, K, gwidth], bf16, tag="hxg")
                            for c0 in range(0, kl, GMAX):
                                ncc = min(GMAX, kl - c0)
                                raw_dma_gather(
                                    nc, hxg[:, c0:c0 + ncc, :],
                                    table[0:lo_end, 0:gwidth],
                                    sidx_sb[:, (t0 + c0) * 8:(t0 + c0 + ncc) * 8],
                                    ncc * P, gwidth, RowW, next_q())
                            for c0 in range(kl, K, GMAX):
                                ncc = min(GMAX, K - c0)
                                raw_dma_gather(
                                    nc, hxg[:, c0:c0 + ncc, :],
                                    table[SPLIT:n_nodes, 0:gwidth],
                                    sidx_sb[:, (t0 + c0) * 8:(t0 + c0 + ncc) * 8],
                                    ncc * P, gwidth, RowW, next_q())
                            # e_dst per edge: edp[p, k*nh:] = edr[slot[p,k], :]
                            edp = ps_edp.tile([P, Kmax * nheads], f32,
                                              tag="edp", space="PSUM")
                            for k in range(K):
                                nc.tensor.matmul(
                                    out=edp[:, k * nheads:(k + 1) * nheads],
                                    lhsT=mtT[:, k * P:(k + 1) * P],
                                    rhs=edr[:, :], start=True, stop=True)
                            tmp = ew.tile([P, K, nheads], f32, tag="tmp")
                            nc.vector.tensor_tensor(
                                out=tmp[:, :, :],
                                in0=hxg[:, :, es_off : es_off + nheads],
                                in1=edp[:, 0:K * nheads].rearrange(
                                    "p (k h) -> p k h", k=K), op=AluOpType.add)
                            # exp(leaky_relu(.)) on the scalar engine
                            nc.scalar.activation(tmp[:, :, :], tmp[:, :, :],
                                                 Prelu, alpha=NEG_SLOPE)
                            ex = ew.tile([P, K, nheads], bf16, tag="ex")
                            nc.scalar.activation(ex[:, :, :], tmp[:, :, :], Exp)
                            rhs = ew.tile([P, K, Racc], bf16, tag="rhs")
                            if nheads == 1:
                                # table row is [h | 1.0 | es]: one multiply
                                # yields [h*ex | ex] including the denominator
                                nc.vector.tensor_tensor(
                                    out=rhs[:, :, :],
                                    in0=hxg[:, :, 0:Racc],
                                    in1=ex[:, :, 0:1].to_broadcast([P, K, Racc]),
                                    op=AluOpType.mult)
                            else:
                                nc.vector.tensor_tensor(
                                    out=rhs[:, :, 0:Cm].rearrange(
                                        "p k (h c) -> p k h c", h=nheads),
                                    in0=hxg[:, :, 0:Cm].rearrange(
                                        "p k (h c) -> p k h c", h=nheads),
                                    in1=ex[:, :, :].to_broadcast(
                                        [P, K, nheads, chead]),
                                    op=AluOpType.mult)
                                nc.vector.tensor_copy(out=rhs[:, :, Cm:Racc],
                                                      in_=ex[:, :, :])
                            acc = ps_acc.tile([P, Racc], f32, tag="acc", space="PSUM")
                            for k in range(K):
                                nc.tensor.matmul(out=acc[:, :],
                                                 lhsT=mt[:, k * P:(k + 1) * P],
                                                 rhs=rhs[:, k, :],
                                                 start=(k == 0), stop=(k == K - 1))
                            flush(b, acc, rows, ew)

                # ----- layer 1 flush: normalize, elu, layer-2 dense, store
                def flush1(b, acc, rows, ew):
                    recip = ew.tile([P, heads], f32, tag="recip")
                    nc.vector.reciprocal(recip[:rows, :], acc[:rows, HC : HC + heads])
                    h1 = ew.tile([P, HC], f32, tag="h1")
                    nc.vector.tensor_tensor(
                        out=h1[:rows, :].rearrange("p (h c) -> p h c", h=heads),
                        in0=acc[:rows, 0:HC].rearrange("p (h c) -> p h c", h=heads),
                        in1=recip[:rows, :].to_broadcast([rows, heads, hid]),
                        op=AluOpType.mult)
                    nc.vector.tensor_add(out=h1[:rows, :], in0=h1[:rows, :],
                                         in1=b1_sb[:rows, :])
                    mn = ew.tile([P, HC], f32, tag="mn")
                    nc.vector.tensor_scalar_min(mn[:rows, :], h1[:rows, :], 0.0)
                    nc.scalar.activation(mn[:rows, :], mn[:rows, :], Exp)
                    mx = ew.tile([P, HC], f32, tag="mx")
                    nc.vector.tensor_scalar_max(mx[:rows, :], h1[:rows, :], 0.0)
                    h1e = ew.tile([P, HC], bf16, tag="h1e")
                    nc.vector.scalar_tensor_tensor(
                        out=h1e[:rows, :], in0=mn[:rows, :], scalar=-1.0,
                        in1=mx[:rows, :], op0=AluOpType.add, op1=AluOpType.add)
                    tp = ew.tile([P, 2, P], bf16, tag="tp")
                    h2p = ps_small.tile([P, W2C], f32, tag="h2p", space="PSUM")
                    for ch in range(2):
                        tps = ps_tr.tile([P, P], bf16, tag="trp", space="PSUM")
                        nc.tensor.transpose(out=tps[:], in_=h1e[:, ch * P:(ch + 1) * P],
                                            identity=ident_sb[:])
                        nc.scalar.copy(out=tp[:, ch, :], in_=tps[:])
                        nc.tensor.matmul(
                            out=h2p[:rows, :],
                            lhsT=tp[:, ch, 0:rows],
                            rhs=w2_sb[:, ch * W2C:(ch + 1) * W2C],
                            start=(ch == 0), stop=(ch == 1))
                    st2 = ew.tile([P, R2], bf16, tag="st2")
                    nc.vector.memset(st2[:rows, G2W:R2], 0.0)
                    nc.vector.memset(st2[:rows, cout:cout + 1], 1.0)
                    nc.scalar.copy(out=st2[:rows, 0:cout], in_=h2p[:rows, 0:cout])
                    nc.scalar.copy(out=st2[:rows, cout + 1:cout + 2],
                                   in_=h2p[:rows, cout:cout + 1])
                    nc.sync.dma_start(out=hx2in[b * P : b * P + rows, :],
                                      in_=st2[:rows, :])
                    se2 = ew.tile([P, 1], bf16, tag="se2")
                    nc.scalar.copy(out=se2[:rows, :],
                                   in_=h2p[:rows, cout + 1:cout + 2])
                    nc.sync.dma_start(out=edt2[b * P : b * P + rows, :],
                                      in_=se2[:rows, :])

                edge_layer(1, hx, R1, G1W, edt, heads, hid, HC, flush1)

                # ----- AllGather hx2 (issued without a pre-barrier; Tile
                # orders it after the hx2in stores via data deps)
                nc.gpsimd.collective_compute(
                    "AllGather", AluOpType.bypass,
                    replica_groups=[list(range(n_cores))],
                    ins=[hx2in[:, :]], outs=[hx2[:, :]])
                tc.strict_bb_all_engine_barrier()

                # ----- layer-2 edge pass (gathers straight from Shared hx2)
                def flush2(b, acc, rows, ew):
                    recip = ew.tile([P, 1], f32, tag="recip2")
                    nc.vector.reciprocal(recip[:rows, :], acc[:rows, cout : cout + 1])
                    o = ew.tile([P, cout], f32, tag="o")
                    nc.vector.tensor_tensor(
                        out=o[:rows, :], in0=acc[:rows, 0:cout],
                        in1=recip[:rows, :].to_broadcast([rows, cout]),
                        op=AluOpType.mult)
                    nc.vector.tensor_add(out=o[:rows, :], in0=o[:rows, :],
                                         in1=b2_sb[:rows, :])
                    nc.sync.dma_start(out=out[b * P : b * P + rows, :],
                                      in_=o[:rows, :])

                edge_layer(2, hx2, R2, G2W, edt2, 1, cout, cout + 1, flush2)

    nc.compile()
    return nc


def host_prep(x, edge_index, W1, a_src1, a_dst1, b1, W2, a_src2, a_dst2, b2,
              n_cores):
    """Plan the edge partition and build per-core input maps."""
    x = np.asarray(x, np.float32)
    n_nodes, cin = x.shape
    heads, hid = np.asarray(a_src1).shape
    cout = np.asarray(W2).shape[1]
    npc = n_nodes // n_cores

    loops = np.arange(n_nodes, dtype=np.int64)
    src = np.concatenate([np.asarray(edge_index[0], np.int64), loops])
    dst = np.concatenate([np.asarray(edge_index[1], np.int64), loops])
    pl = plan(src, dst, n_nodes, n_cores)

    W1 = np.asarray(W1, np.float32)
    W1h = W1.reshape(cin, heads, hid)
    Wa1 = np.einsum("khc,hc->kh", W1h, np.asarray(a_src1, np.float32))
    Wb1 = np.einsum("khc,hc->kh", W1h, np.asarray(a_dst1, np.float32))
    Wx1 = np.concatenate([W1, Wa1], axis=1).astype(BF)

    W2 = np.asarray(W2, np.float32)
    Wa2 = (W2 * np.asarray(a_src2, np.float32)).sum(1, keepdims=True)
    Wb2 = (W2 * np.asarray(a_dst2, np.float32)).sum(1, keepdims=True)
    W2e = np.concatenate([W2, Wa2, Wb2], axis=1)
    Wx2 = np.ascontiguousarray(
        np.concatenate([W2e[:P], W2e[P:]], axis=1)).astype(BF)

    xTb = np.ascontiguousarray(x.T).astype(BF)
    common = {
        "xT": xTb,
        "Wx1": Wx1,
        "Wb1": np.ascontiguousarray(Wb1).astype(BF),
        "Wx2": Wx2,
        "b1r": np.tile(np.asarray(b1, np.float32)[None, :], (P, 1)),
        "b2r": np.tile(np.asarray(b2, np.float32)[None, :], (P, 1)),
        "identb": np.eye(P, dtype=np.float32).astype(BF),
    }
    in_maps = []
    for c in range(n_cores):
        m = dict(common)
        m["xTl"] = np.ascontiguousarray(xTb[:, c * npc:(c + 1) * npc])
        m["srcw"] = np.ascontiguousarray(pl["srcw"][c])
        m["mtc"] = np.ascontiguousarray(pl["mtc"][c])
        in_maps.append(m)
    return pl, (n_nodes, cin, heads, hid, cout), in_maps


def run_gat(x, edge_index, W1, a_src1, a_dst1, b1, W2, a_src2, a_dst2, b2,
            n_cores=8, trace=False):
    pl, (n_nodes, cin, heads, hid, cout), in_maps = host_prep(
        x, edge_index, W1, a_src1, a_dst1, b1, W2, a_src2, a_dst2, b2, n_cores)
    nc = build(pl, n_nodes, cin, heads, hid, cout, n_cores)
    res = bass_utils.run_bass_kernel_spmd(
        nc, in_maps, core_ids=list(range(n_cores)), trace=trace)
    outp = np.concatenate([res.results[c]["out"] for c in range(n_cores)], axis=0)
    return outp[:n_nodes], res


def kernel(**inputs):
    """Full-input GAT kernel: shards internally across 8 NeuronCores."""
    x = np.asarray(inputs["x"], np.float32)
    edge_index = np.asarray(inputs["edge_index"])
    outp, _ = run_gat(
        x, edge_index,
        inputs["W1"], inputs["a_src1"], inputs["a_dst1"], inputs["b1"],
        inputs["W2"], inputs["a_src2"], inputs["a_dst2"], inputs["b2"],
        n_cores=8, trace=bool(int(os.environ.get("GAT_TRACE", "0"))))
    return outp.astype(np.float32)


# revision 27
# speedup vs baseline: 1.8233x; 1.0380x over previous
"""GAT (2-layer, PyG-style) on 8 Trainium2 NeuronCores via Bass/Tile.

v4: dst-sharded nodes+edges across 8 cores.
 - dma_gather instructions round-robin over 4 SWDGE queues (4 Q7 core
   pairs emit descriptors concurrently; 3.9x emission throughput).
 - e_dst per edge via one-hot matmuls: host-precomputed fp8 one-hot
   matrices (mtT for dst->edge broadcast, mt for edge->dst segment sum)
   stream from DRAM; fp8 lhsT x bf16 rhs matmuls are exact for 0/1
   weights.  No PE transposes, no vector is_eq.
 - phase 1 stores hx rows as full contiguous 768B rows (few large DMA
   descriptors instead of 128 strided ones per block).
 - layer-2 table is gathered directly from the AllGather Shared-space
   output (no Shared->Local bounce copy).
The same index/one-hot arrays drive both layers (identical edge plan).
"""
import os
import sys

sys.path.insert(0, "/opt/trn_rl_repo")

import numpy as np
import ml_dtypes

import concourse.bass as bass
import concourse.mybir as mybir
import concourse.tile as tile
from concourse import bacc, bass_utils, library_config
from concourse.alu_op_type import AluOpType

P = 128
NEG_SLOPE = 0.2
GMAX = 8            # max idx columns per dma_gather = 1024 idx (HW limit)
BF = ml_dtypes.bfloat16
F8 = ml_dtypes.float8_e4m3
SPLIT = 32768       # int16 idx limit for dma_gather


def wrap_idx(vals):
    """idx sequence (len%128==0, 0<=v<32768) -> wrapped [128, len//16] int16."""
    a = np.asarray(vals, np.int64)
    assert len(a) % 128 == 0 and a.min() >= 0 and a.max() < SPLIT
    w = a.reshape(-1, 16).T.astype(np.int16)
    return np.tile(w, (8, 1))


def plan(src, dst, n_nodes, n_cores):
    npc = n_nodes // n_cores
    nblk = (npc + P - 1) // P
    order = np.argsort(dst, kind="stable")
    src_s = src[order].astype(np.int64)
    dst_s = dst[order].astype(np.int64)

    per = []            # [core][block] = (src_lo, src_hi, dst_lo_lo, dst_lo_hi)
    KL = np.zeros(nblk, np.int64)
    KH = np.zeros(nblk, np.int64)
    for c in range(n_cores):
        base = c * npc
        rows = []
        for b in range(nblk):
            n0 = base + b * P
            n1 = base + min((b + 1) * P, npc)
            e0 = np.searchsorted(dst_s, n0, side="left")
            e1 = np.searchsorted(dst_s, n1, side="left")
            s = src_s[e0:e1]
            dl = dst_s[e0:e1] - base          # core-local dst
            m = s < SPLIT
            rows.append((s[m], s[~m] - SPLIT, dl[m], dl[~m]))
            KL[b] = max(KL[b], (m.sum() + P - 1) // P)
            KH[b] = max(KH[b], ((~m).sum() + P - 1) // P)
        per.append(rows)

    Kb = KL + KH
    tob = np.concatenate([[0], np.cumsum(Kb)]).astype(np.int64)
    Ttot = int(Kb.sum())
    srcw = np.zeros((n_cores, P, Ttot * 8), np.int16)
    slot = np.full((n_cores, P, Ttot), -1, np.int64)
    for c in range(n_cores):
        for b in range(nblk):
            slo, shi, dlo, dhi = per[c][b]
            kl, kh = int(KL[b]), int(KH[b])
            t0 = int(tob[b])
            n0b = b * P
            for ss, dd, K, toff in [(slo, dlo, kl, t0), (shi, dhi, kh, t0 + kl)]:
                n = len(ss)
                npad = K * P
                if npad == 0:
                    continue
                a = np.zeros(npad, np.int64)
                a[:n] = ss
                sl = np.full(npad, -1, np.int64)
                sl[:n] = (dd - n0b)
                srcw[c, :, toff * 8:(toff + K) * 8] = wrap_idx(a)
                slot[c, :, toff:toff + K] = sl.reshape(K, P).T
    # mtT one-hot (fp8): mtT[q, t*128+p] = (slot[p,t]==q), the lhsT for the
    # dst->edge e_dst broadcast matmul.  Stored block-contiguous ([P, K*P]
    # row-major per block) so each block's load is one sequential DRAM read.
    # mt (the segment-sum lhsT) is built on-device by vector is_eq from
    # slot + iota, so only half the one-hot volume streams from HBM.
    mtTb = np.zeros((n_cores, P, Ttot * P), np.uint8)
    one = np.float32(1.0).astype(F8).view(np.uint8)
    for c in range(n_cores):
        pp, tt = np.nonzero(slot[c] >= 0)
        qq = slot[c][pp, tt]
        mtTb[c, qq, tt * P + pp] = one
    mtc = np.zeros((n_cores, Ttot * P * P), np.uint8)
    for c in range(n_cores):
        off = 0
        for b in range(nblk):
            t0, t1 = int(tob[b]), int(tob[b + 1])
            w = (t1 - t0) * P
            mtc[c, off:off + w * P] = mtTb[c][:, t0 * P:t1 * P].reshape(-1)
            off += w * P
    return dict(npc=npc, nblk=nblk, KL=KL, KH=KH, Kb=Kb, tob=tob, Ttot=Ttot,
                Kmax=int(Kb.max()), srcw=srcw, mtc=mtc.view(F8),
                slot=slot.astype(np.float32))


def raw_dma_gather(nc, out_ap, in_ap, idxs_ap, num_idxs, elem_size, elem_step,
                   queue_num):
    """dma_gather (non-transpose, DRAM source) without the %256 payload
    restriction; row stride (elem_step elements) must be a 256B multiple."""
    g = nc.gpsimd
    stride_bytes = elem_step * mybir.dt.size(in_ap.dtype)
    sb256 = stride_bytes // 256
    assert stride_bytes % 256 == 0 and sb256 < 256
    _in_ap = g.lower_ap_dma(in_ap, for_custom_bir_dma=True)
    _idxs_ap = g.lower_ap(idxs_ap)
    _out_ap = g.lower_ap(out_ap)
    return g.add_instruction(
        mybir.InstDMAGatherAnt(
            name=g.bass.get_next_instruction_name(),
            ins=[*_in_ap, _idxs_ap, g.lower_val_access(g.to_reg(num_idxs))],
            outs=[_out_ap], transpose=False, num_idxs=num_idxs,
            elem_size=elem_size, stride_bytes_256=sb256, gen_mode=0,
            single_packet=True, queue_num=queue_num, sbuf_tokens_per_rank=0,
            sbuf_free_dim_per_rank=0, sbuf_free_dim_pad_per_rank=0,
            sbuf_byte_offset=0))


def build(pl, n_nodes, cin, heads, hid, cout, n_cores):
    HC = heads * hid            # 256
    G1W = HC + heads            # 264 = [h | es] gather payload
    R1 = 384                    # hx row stride (768B)
    G2W = cout + 2              # 66  = [h2 | 1.0 | es2] gather payload
    R2 = 128                    # hx2 row stride (256B)
    npc, nblk = pl["npc"], pl["nblk"]
    KL, KH, Kb, tob = pl["KL"], pl["KH"], pl["Kb"], pl["tob"]
    Ttot, Kmax = pl["Ttot"], pl["Kmax"]
    NT1 = (n_nodes + P - 1) // P

    nc = bacc.Bacc("TRN2", num_swdge_queues=4)
    f32 = mybir.dt.float32
    bf16 = mybir.dt.bfloat16
    fp8 = mybir.dt.float8e4
    i16 = mybir.dt.int16
    Exp = mybir.ActivationFunctionType.Exp

    xT = nc.dram_tensor("xT", [cin, n_nodes], bf16, kind="ExternalInput")
    xTl = nc.dram_tensor("xTl", [cin, npc], bf16, kind="ExternalInput")
    Wx1 = nc.dram_tensor("Wx1", [cin, G1W], bf16, kind="ExternalInput")
    Wb1 = nc.dram_tensor("Wb1", [cin, heads], bf16, kind="ExternalInput")
    Wx2 = nc.dram_tensor("Wx2", [P, 2 * (cout + 2)], bf16, kind="ExternalInput")
    b1r = nc.dram_tensor("b1r", [P, HC], f32, kind="ExternalInput")
    b2r = nc.dram_tensor("b2r", [P, cout], f32, kind="ExternalInput")
    identb = nc.dram_tensor("identb", [P, P], bf16, kind="ExternalInput")
    srcw = nc.dram_tensor("srcw", [P, Ttot * 8], i16, kind="ExternalInput")
    mtc = nc.dram_tensor("mtc", [Ttot * P * P], fp8, kind="ExternalInput")
    iota_rep = nc.dram_tensor("iota_rep", [P, Kmax * P], bf16, kind="ExternalInput")
    slotb = nc.dram_tensor("slotb", [P, Ttot], bf16, kind="ExternalInput")
    out = nc.dram_tensor("out", [npc, cout], f32, kind="ExternalOutput")

    hx = nc.dram_tensor("hx", [n_nodes, R1], bf16)
    edt = nc.dram_tensor("edt", [npc, heads], bf16)
    edt2 = nc.dram_tensor("edt2", [npc, 1], bf16)
    hx2in = nc.dram_tensor("hx2in", [npc, R2], bf16)
    hx2 = nc.dram_tensor("hx2", [n_nodes, R2], bf16, addr_space="Shared")

    W2C = cout + 2              # 66 = [h2 | es2 | ed2] from the flush matmul
    Prelu = mybir.ActivationFunctionType.Prelu

    qctr = [0]

    def next_q():
        q = qctr[0] & 3
        qctr[0] += 1
        return q

    with tile.TileContext(nc) as tc:
        with tc.tile_pool(name="const", bufs=1) as cp:
            nc.gpsimd.load_library(library_config.mlp)
            ident_sb = cp.tile([P, P], bf16)
            b1_sb = cp.tile([P, HC], f32)
            b2_sb = cp.tile([P, cout], f32)
            w1_sb = cp.tile([cin, G1W], bf16)
            wb1_sb = cp.tile([cin, heads], bf16)
            w2_sb = cp.tile([P, 2 * W2C], bf16)
            sidx_sb = cp.tile([P, Ttot * 8], i16)
            iota_sb = cp.tile([P, Kmax, P], bf16)
            slot_sb = cp.tile([P, Ttot], bf16)
            nc.sync.dma_start(
                out=iota_sb[:, :, :],
                in_=iota_rep[:, :].rearrange("p (k q) -> p k q", q=P))
            nc.sync.dma_start(out=slot_sb[:], in_=slotb[:, :])
            nc.sync.dma_start(out=ident_sb[:], in_=identb[:, :])
            nc.sync.dma_start(out=b1_sb[:], in_=b1r[:, :])
            nc.sync.dma_start(out=b2_sb[:], in_=b2r[:, :])
            nc.sync.dma_start(out=w1_sb[:], in_=Wx1[:, :])
            nc.sync.dma_start(out=wb1_sb[:], in_=Wb1[:, :])
            nc.sync.dma_start(out=w2_sb[:], in_=Wx2[:, :])
            nc.sync.dma_start(out=sidx_sb[:], in_=srcw[:, :])

            # ---------- phase 1: hx = [x@W1 | es] bf16 for ALL nodes
            # (contiguous full-row stores); edt = x_local@Wb1 for LOCAL nodes
            XB = 8
            with tc.tile_pool(name="p1", bufs=3) as p1, \
                 tc.tile_pool(name="p1ps", bufs=4, space="PSUM") as p1ps:
                for j0 in range(0, NT1, XB):
                    jn = min(XB, NT1 - j0)
                    w = min(XB * P, n_nodes - j0 * P)
                    xt_sb = p1.tile([cin, XB * P], bf16, tag="xt")
                    nc.sync.dma_start(out=xt_sb[:, :w], in_=xT[:, j0 * P : j0 * P + w])
                    st = p1.tile([P, XB, R1], bf16, tag="st")
                    for k in range(jn):
                        rows = min(P, n_nodes - (j0 + k) * P)
                        ps = p1ps.tile([P, G1W], f32, tag="ps")
                        nc.tensor.matmul(out=ps[:rows, :],
                                         lhsT=xt_sb[:, k * P : k * P + rows],
                                         rhs=w1_sb[:, :], start=True, stop=True)
                        if k % 2 == 0:
                            nc.scalar.copy(out=st[:rows, k, 0:G1W], in_=ps[:rows, :])
                        else:
                            nc.vector.tensor_copy(out=st[:rows, k, 0:G1W], in_=ps[:rows, :])
                    full = jn if w == jn * P else jn - 1
                    if full:
                        nc.sync.dma_start(
                            out=hx[j0 * P : (j0 + full) * P, :].rearrange(
                                "(a b) c -> b a c", b=P),
                            in_=st[:, 0:full, :])
                    if full < jn:
                        rows = w - full * P
                        nc.sync.dma_start(
                            out=hx[(j0 + full) * P : j0 * P + w, 0:G1W],
                            in_=st[:rows, full, 0:G1W])
                for j0 in range(0, nblk, XB):
                    jn = min(XB, nblk - j0)
                    w = min(XB * P, npc - j0 * P)
                    xt_sb = p1.tile([cin, XB * P], bf16, tag="xt")
                    nc.sync.dma_start(out=xt_sb[:, :w], in_=xTl[:, j0 * P : j0 * P + w])
                    ste = p1.tile([P, XB, heads], bf16, tag="ste")
                    for k in range(jn):
                        rows = min(P, npc - (j0 + k) * P)
                        pse = p1ps.tile([P, heads], f32, tag="pse")
                        nc.tensor.matmul(out=pse[:rows, :],
                                         lhsT=xt_sb[:, k * P : k * P + rows],
                                         rhs=wb1_sb[:, :], start=True, stop=True)
                        if k % 2 == 0:
                            nc.scalar.copy(out=ste[:rows, k, :], in_=pse[:rows, :])
                        else:
                            nc.vector.tensor_copy(out=ste[:rows, k, :], in_=pse[:rows, :])
                    full = jn if w == jn * P else jn - 1
                    if full:
                        nc.sync.dma_start(
                            out=edt[j0 * P : (j0 + full) * P, :].rearrange(
                                "(a b) c -> b a c", b=P),
                            in_=ste[:, 0:full, :])
                    if full < jn:
                        rows = w - full * P
                        nc.sync.dma_start(
                            out=edt[(j0 + full) * P : j0 * P + w, :],
                            in_=ste[:rows, full, :])

            tc.strict_bb_all_engine_barrier()

            with tc.tile_pool(name="ps_acc", bufs=2, space="PSUM") as ps_acc, \
                 tc.tile_pool(name="ps_tr", bufs=2, space="PSUM") as ps_tr, \
                 tc.tile_pool(name="ps_small", bufs=2, space="PSUM") as ps_small, \
                 tc.tile_pool(name="ps_edp", bufs=2, space="PSUM") as ps_edp:

                def edge_layer(layer, table, RowW, gwidth, edtab, nheads,
                               chead, es_off, flush):
                    Cm = nheads * chead
                    Racc = Cm + nheads
                    lo_end = min(SPLIT, n_nodes)
                    with tc.tile_pool(name=f"eg{layer}", bufs=4) as eg, \
                         tc.tile_pool(name=f"ew{layer}", bufs=3) as ew, \
                         tc.tile_pool(name=f"em{layer}", bufs=3) as em:
                        for b in range(nblk):
                            kl, kh = int(KL[b]), int(KH[b])
                            K = kl + kh
                            t0 = int(tob[b])
                            rows = min(P, npc - b * P)
                            # per-block e_dst rows: one contiguous DMA
                            edr = ew.tile([P, nheads], bf16, tag="edr")
                            if rows < P:
                                nc.vector.memset(edr[:, :], 0.0)
                            nc.sync.dma_start(
                                out=edr[:rows, :],
                                in_=edtab[b * P : b * P + rows, :])
                            # mtT one-hot: one sequential read; mt via is_eq
                            mtT = em.tile([P, Kmax * P], fp8, tag="mtT")
                            o0 = t0 * P * P
                            nc.sync.dma_start(
                                out=mtT[:, 0:K * P],
                                in_=mtc[o0:o0 + K * P * P].rearrange(
                                    "(p c) -> p c", p=P))
                            mt = em.tile([P, Kmax, P], bf16, tag="mt")
                            nc.vector.tensor_tensor(
                                out=mt[:, 0:K, :], in0=iota_sb[:, 0:K, :],
                                in1=slot_sb[:, t0:t0 + K].to_broadcast(
                                    [P, K, P]),
                                op=AluOpType.is_equal)
                            hxg = eg.tile([P, K, gwidth], bf16, tag="hxg")
                            # balanced chunking: equal-size gather instrs so
                            # all 4 SWDGE queue pairs carry equal emission
                            for base, n, tab in ((0, kl, table[0:lo_end, 0:gwidth]),
                                                 (kl, kh, table[SPLIT:n_nodes, 0:gwidth])):
                                if n == 0:
                                    continue
                                parts = 1 if n == 1 else max(2, (n + GMAX - 1) // GMAX)
                                c0 = base
                                for i in range(parts):
                                    ncc = n // parts + (1 if i < n % parts else 0)
                                    raw_dma_gather(
                                        nc, hxg[:, c0:c0 + ncc, :], tab,
                                        sidx_sb[:, (t0 + c0) * 8:(t0 + c0 + ncc) * 8],
                                        ncc * P, gwidth, RowW, next_q())
                                    c0 += ncc
                            # e_dst per edge: edp[p, k*nh:] = edr[slot[p,k], :]
                            edp = ps_edp.tile([P, Kmax * nheads], f32,
                                              tag="edp", space="PSUM")
                            for k in range(K):
                                nc.tensor.matmul(
                                    out=edp[:, k * nheads:(k + 1) * nheads],
                                    lhsT=mtT[:, k * P:(k + 1) * P],
                                    rhs=edr[:, :], start=True, stop=True)
                            tmp = ew.tile([P, K, nheads], f32, tag="tmp")
                            nc.vector.tensor_tensor(
                                out=tmp[:, :, :],
                                in0=hxg[:, :, es_off : es_off + nheads],
                                in1=edp[:, 0:K * nheads].rearrange(
                                    "p (k h) -> p k h", k=K), op=AluOpType.add)
                            # exp(leaky_relu(.)) on the scalar engine
                            nc.scalar.activation(tmp[:, :, :], tmp[:, :, :],
                                                 Prelu, alpha=NEG_SLOPE)
                            ex = ew.tile([P, K, nheads], bf16, tag="ex")
                            nc.scalar.activation(ex[:, :, :], tmp[:, :, :], Exp)
                            rhs = ew.tile([P, K, Racc], bf16, tag="rhs")
                            if nheads == 1:
                                # table row is [h | 1.0 | es]: one multiply
                                # yields [h*ex | ex] including the denominator
                                nc.vector.tensor_tensor(
                                    out=rhs[:, :, :],
                                    in0=hxg[:, :, 0:Racc],
                                    in1=ex[:, :, 0:1].to_broadcast([P, K, Racc]),
                                    op=AluOpType.mult)
                            else:
                                nc.vector.tensor_tensor(
                                    out=rhs[:, :, 0:Cm].rearrange(
                                        "p k (h c) -> p k h c", h=nheads),
                                    in0=hxg[:, :, 0:Cm].rearrange(
                                        "p k (h c) -> p k h c", h=nheads),
                                    in1=ex[:, :, :].to_broadcast(
                                        [P, K, nheads, chead]),
                                    op=AluOpType.mult)
                                nc.vector.tensor_copy(out=rhs[:, :, Cm:Racc],
                                                      in_=ex[:, :, :])
                            acc = ps_acc.tile([P, Racc], f32, tag="acc", space="PSUM")
                            for k in range(K):
                                nc.tensor.matmul(out=acc[:, :],
                                                 lhsT=mt[:, k, :],
                                                 rhs=rhs[:, k, :],
                                                 start=(k == 0), stop=(k == K - 1))
                            flush(b, acc, rows, ew)

                # ----- layer 1 flush: normalize, elu, layer-2 dense, store
                def flush1(b, acc, rows, ew):
                    recip = ew.tile([P, heads], f32, tag="recip")
                    nc.vector.reciprocal(recip[:rows, :], acc[:rows, HC : HC + heads])
                    h1 = ew.tile([P, HC], f32, tag="h1")
                    nc.vector.tensor_tensor(
                        out=h1[:rows, :].rearrange("p (h c) -> p h c", h=heads),
                        in0=acc[:rows, 0:HC].rearrange("p (h c) -> p h c", h=heads),
                        in1=recip[:rows, :].to_broadcast([rows, heads, hid]),
                        op=AluOpType.mult)
                    nc.vector.tensor_add(out=h1[:rows, :], in0=h1[:rows, :],
                                         in1=b1_sb[:rows, :])
                    mn = ew.tile([P, HC], f32, tag="mn")
                    nc.vector.tensor_scalar_min(mn[:rows, :], h1[:rows, :], 0.0)
                    nc.scalar.activation(mn[:rows, :], mn[:rows, :], Exp)
                    mx = ew.tile([P, HC], f32, tag="mx")
                    nc.vector.tensor_scalar_max(mx[:rows, :], h1[:rows, :], 0.0)
                    h1e = ew.tile([P, HC], bf16, tag="h1e")
                    nc.vector.scalar_tensor_tensor(
                        out=h1e[:rows, :], in0=mn[:rows, :], scalar=-1.0,
                        in1=mx[:rows, :], op0=AluOpType.add, op1=AluOpType.add)
                    tp = ew.tile([P, 2, P], bf16, tag="tp")
                    h2p = ps_small.tile([P, W2C], f32, tag="h2p", space="PSUM")
                    for ch in range(2):
                        tps = ps_tr.tile([P, P], bf16, tag="trp", space="PSUM")
                        nc.tensor.transpose(out=tps[:], in_=h1e[:, ch * P:(ch + 1) * P],
                                            identity=ident_sb[:])
                        nc.scalar.copy(out=tp[:, ch, :], in_=tps[:])
                        nc.tensor.matmul(
                            out=h2p[:rows, :],
                            lhsT=tp[:, ch, 0:rows],
                            rhs=w2_sb[:, ch * W2C:(ch + 1) * W2C],
                            start=(ch == 0), stop=(ch == 1))
                    st2 = ew.tile([P, R2], bf16, tag="st2")
                    nc.vector.memset(st2[:rows, G2W:R2], 0.0)
                    nc.vector.memset(st2[:rows, cout:cout + 1], 1.0)
                    nc.scalar.copy(out=st2[:rows, 0:cout], in_=h2p[:rows, 0:cout])
                    nc.scalar.copy(out=st2[:rows, cout + 1:cout + 2],
                                   in_=h2p[:rows, cout:cout + 1])
                    nc.sync.dma_start(out=hx2in[b * P : b * P + rows, :],
                                      in_=st2[:rows, :])
                    se2 = ew.tile([P, 1], bf16, tag="se2")
                    nc.scalar.copy(out=se2[:rows, :],
                                   in_=h2p[:rows, cout + 1:cout + 2])
                    nc.sync.dma_start(out=edt2[b * P : b * P + rows, :],
                                      in_=se2[:rows, :])

                edge_layer(1, hx, R1, G1W, edt, heads, hid, HC, flush1)

                # ----- AllGather hx2 (issued without a pre-barrier; Tile
                # orders it after the hx2in stores via data deps)
                nc.gpsimd.collective_compute(
                    "AllGather", AluOpType.bypass,
                    replica_groups=[list(range(n_cores))],
                    ins=[hx2in[:, :]], outs=[hx2[:, :]])
                tc.strict_bb_all_engine_barrier()

                # ----- layer-2 edge pass (gathers straight from Shared hx2)
                def flush2(b, acc, rows, ew):
                    recip = ew.tile([P, 1], f32, tag="recip2")
                    nc.vector.reciprocal(recip[:rows, :], acc[:rows, cout : cout + 1])
                    o = ew.tile([P, cout], f32, tag="o")
                    nc.vector.tensor_tensor(
                        out=o[:rows, :], in0=acc[:rows, 0:cout],
                        in1=recip[:rows, :].to_broadcast([rows, cout]),
                        op=AluOpType.mult)
                    nc.vector.tensor_add(out=o[:rows, :], in0=o[:rows, :],
                                         in1=b2_sb[:rows, :])
                    nc.sync.dma_start(out=out[b * P : b * P + rows, :],
                                      in_=o[:rows, :])

                edge_layer(2, hx2, R2, G2W, edt2, 1, cout, cout + 1, flush2)

    nc.compile()
    return nc


def host_prep(x, edge_index, W1, a_src1, a_dst1, b1, W2, a_src2, a_dst2, b2,
              n_cores):
    """Plan the edge partition and build per-core input maps."""
    x = np.asarray(x, np.float32)
    n_nodes, cin = x.shape
    heads, hid = np.asarray(a_src1).shape
    cout = np.asarray(W2).shape[1]
    npc = n_nodes // n_cores

    loops = np.arange(n_nodes, dtype=np.int64)
    src = np.concatenate([np.asarray(edge_index[0], np.int64), loops])
    dst = np.concatenate([np.asarray(edge_index[1], np.int64), loops])
    pl = plan(src, dst, n_nodes, n_cores)

    W1 = np.asarray(W1, np.float32)
    W1h = W1.reshape(cin, heads, hid)
    Wa1 = np.einsum("khc,hc->kh", W1h, np.asarray(a_src1, np.float32))
    Wb1 = np.einsum("khc,hc->kh", W1h, np.asarray(a_dst1, np.float32))
    Wx1 = np.concatenate([W1, Wa1], axis=1).astype(BF)

    W2 = np.asarray(W2, np.float32)
    Wa2 = (W2 * np.asarray(a_src2, np.float32)).sum(1, keepdims=True)
    Wb2 = (W2 * np.asarray(a_dst2, np.float32)).sum(1, keepdims=True)
    W2e = np.concatenate([W2, Wa2, Wb2], axis=1)
    Wx2 = np.ascontiguousarray(
        np.concatenate([W2e[:P], W2e[P:]], axis=1)).astype(BF)

    xTb = np.ascontiguousarray(x.T).astype(BF)
    common = {
        "xT": xTb,
        "Wx1": Wx1,
        "Wb1": np.ascontiguousarray(Wb1).astype(BF),
        "Wx2": Wx2,
        "b1r": np.tile(np.asarray(b1, np.float32)[None, :], (P, 1)),
        "b2r": np.tile(np.asarray(b2, np.float32)[None, :], (P, 1)),
        "identb": np.eye(P, dtype=np.float32).astype(BF),
        "iota_rep": np.tile(np.arange(P, dtype=np.float32),
                            (P, pl["Kmax"])).astype(BF),
    }
    in_maps = []
    for c in range(n_cores):
        m = dict(common)
        m["xTl"] = np.ascontiguousarray(xTb[:, c * npc:(c + 1) * npc])
        m["srcw"] = np.ascontiguousarray(pl["srcw"][c])
        m["mtc"] = np.ascontiguousarray(pl["mtc"][c])
        m["slotb"] = np.ascontiguousarray(pl["slot"][c]).astype(BF)
        in_maps.append(m)
    return pl, (n_nodes, cin, heads, hid, cout), in_maps


def run_gat(x, edge_index, W1, a_src1, a_dst1, b1, W2, a_src2, a_dst2, b2,
            n_cores=8, trace=False):
    pl, (n_nodes, cin, heads, hid, cout), in_maps = host_prep(
        x, edge_index, W1, a_src1, a_dst1, b1, W2, a_src2, a_dst2, b2, n_cores)
    nc = build(pl, n_nodes, cin, heads, hid, cout, n_cores)
    res = bass_utils.run_bass_kernel_spmd(
        nc, in_maps, core_ids=list(range(n_cores)), trace=trace)
    outp = np.concatenate([res.results[c]["out"] for c in range(n_cores)], axis=0)
    return outp[:n_nodes], res


def kernel(**inputs):
    """Full-input GAT kernel: shards internally across 8 NeuronCores."""
    x = np.asarray(inputs["x"], np.float32)
    edge_index = np.asarray(inputs["edge_index"])
    outp, _ = run_gat(
        x, edge_index,
        inputs["W1"], inputs["a_src1"], inputs["a_dst1"], inputs["b1"],
        inputs["W2"], inputs["a_src2"], inputs["a_dst2"], inputs["b2"],
        n_cores=8, trace=bool(int(os.environ.get("GAT_TRACE", "0"))))
    return outp.astype(np.float32)


# revision 31
# speedup vs baseline: 1.8393x; 1.0088x over previous
"""GAT (2-layer, PyG-style) on 8 Trainium2 NeuronCores via Bass/Tile.

v4: dst-sharded nodes+edges across 8 cores.
 - dma_gather instructions round-robin over 4 SWDGE queues (4 Q7 core
   pairs emit descriptors concurrently; 3.9x emission throughput).
 - e_dst per edge via one-hot matmuls: host-precomputed fp8 one-hot
   matrices (mtT for dst->edge broadcast, mt for edge->dst segment sum)
   stream from DRAM; fp8 lhsT x bf16 rhs matmuls are exact for 0/1
   weights.  No PE transposes, no vector is_eq.
 - phase 1 stores hx rows as full contiguous 768B rows (few large DMA
   descriptors instead of 128 strided ones per block).
 - layer-2 table is gathered directly from the AllGather Shared-space
   output (no Shared->Local bounce copy).
The same index/one-hot arrays drive both layers (identical edge plan).
"""
import os
import sys

sys.path.insert(0, "/opt/trn_rl_repo")

import numpy as np
import ml_dtypes

import concourse.bass as bass
import concourse.mybir as mybir
import concourse.tile as tile
from concourse import bacc, bass_utils, library_config
from concourse.alu_op_type import AluOpType

P = 128
NEG_SLOPE = 0.2
GMAX = 8            # max idx columns per dma_gather = 1024 idx (HW limit)
BF = ml_dtypes.bfloat16
F8 = ml_dtypes.float8_e4m3
SPLIT = 32768       # int16 idx limit for dma_gather


def wrap_idx(vals):
    """idx sequence (len%128==0, -1 pads allowed) -> wrapped [128, len//16]
    int16, replicated across the 8 Q7 core pairs."""
    a = np.asarray(vals, np.int64)
    assert len(a) % 128 == 0 and a.min() >= -1 and a.max() < SPLIT
    w = a.reshape(-1, 16).T.astype(np.int16)
    return np.tile(w, (8, 1))


def plan(src, dst, n_nodes, n_cores):
    npc = n_nodes // n_cores
    nblk = (npc + P - 1) // P
    order = np.argsort(dst, kind="stable")
    src_s = src[order].astype(np.int64)
    dst_s = dst[order].astype(np.int64)

    per = []            # [core][block] = (src_lo, src_hi, dst_lo_lo, dst_lo_hi)
    KL = np.zeros(nblk, np.int64)
    KH = np.zeros(nblk, np.int64)
    for c in range(n_cores):
        base = c * npc
        rows = []
        for b in range(nblk):
            n0 = base + b * P
            n1 = base + min((b + 1) * P, npc)
            e0 = np.searchsorted(dst_s, n0, side="left")
            e1 = np.searchsorted(dst_s, n1, side="left")
            s = src_s[e0:e1]
            dl = dst_s[e0:e1] - base          # core-local dst
            m = s < SPLIT
            rows.append((s[m], s[~m] - SPLIT, dl[m], dl[~m]))
            KL[b] = max(KL[b], (m.sum() + P - 1) // P)
            KH[b] = max(KH[b], ((~m).sum() + P - 1) // P)
        per.append(rows)

    Kb = KL + KH
    tob = np.concatenate([[0], np.cumsum(Kb)]).astype(np.int64)
    Ttot = int(Kb.sum())
    srcw = np.zeros((n_cores, P, Ttot * 8), np.int16)
    slot = np.full((n_cores, P, Ttot), -1, np.int64)
    for c in range(n_cores):
        for b in range(nblk):
            slo, shi, dlo, dhi = per[c][b]
            kl, kh = int(KL[b]), int(KH[b])
            t0 = int(tob[b])
            n0b = b * P
            for ss, dd, K, toff in [(slo, dlo, kl, t0), (shi, dhi, kh, t0 + kl)]:
                n = len(ss)
                npad = K * P
                if npad == 0:
                    continue
                # sort by source for HBM locality; the slot map absorbs
                # the permutation.  Pads gather row 0 (harmless).
                order = np.argsort(ss, kind="stable")
                a = np.zeros(npad, np.int64)
                a[:n] = ss[order]
                sl = np.full(npad, -1, np.int64)
                sl[:n] = dd[order] - n0b
                srcw[c, :, toff * 8:(toff + K) * 8] = wrap_idx(a)
                slot[c, :, toff:toff + K] = sl.reshape(K, P).T
    # mtT one-hot (fp8): mtT[q, t*128+p] = (slot[p,t]==q), the lhsT for the
    # dst->edge e_dst broadcast matmul.  Stored block-contiguous ([P, K*P]
    # row-major per block) so each block's load is one sequential DRAM read.
    # mt (the segment-sum lhsT) is built on-device by vector is_eq from
    # slot + iota, so only half the one-hot volume streams from HBM.
    mtTb = np.zeros((n_cores, P, Ttot * P), np.uint8)
    one = np.float32(1.0).astype(F8).view(np.uint8)
    for c in range(n_cores):
        pp, tt = np.nonzero(slot[c] >= 0)
        qq = slot[c][pp, tt]
        mtTb[c, qq, tt * P + pp] = one
    mtc = np.zeros((n_cores, Ttot * P * P), np.uint8)
    for c in range(n_cores):
        off = 0
        for b in range(nblk):
            t0, t1 = int(tob[b]), int(tob[b + 1])
            w = (t1 - t0) * P
            mtc[c, off:off + w * P] = mtTb[c][:, t0 * P:t1 * P].reshape(-1)
            off += w * P
    return dict(npc=npc, nblk=nblk, KL=KL, KH=KH, Kb=Kb, tob=tob, Ttot=Ttot,
                Kmax=int(Kb.max()), srcw=srcw, mtc=mtc.view(F8),
                slot=slot.astype(np.float32))


def raw_dma_gather(nc, out_ap, in_ap, idxs_ap, num_idxs, elem_size, elem_step,
                   queue_num):
    """dma_gather (non-transpose, DRAM source) without the %256 payload
    restriction; row stride (elem_step elements) must be a 256B multiple."""
    g = nc.gpsimd
    stride_bytes = elem_step * mybir.dt.size(in_ap.dtype)
    sb256 = stride_bytes // 256
    assert stride_bytes % 256 == 0 and sb256 < 256
    _in_ap = g.lower_ap_dma(in_ap, for_custom_bir_dma=True)
    _idxs_ap = g.lower_ap(idxs_ap)
    _out_ap = g.lower_ap(out_ap)
    return g.add_instruction(
        mybir.InstDMAGatherAnt(
            name=g.bass.get_next_instruction_name(),
            ins=[*_in_ap, _idxs_ap, g.lower_val_access(g.to_reg(num_idxs))],
            outs=[_out_ap], transpose=False, num_idxs=num_idxs,
            elem_size=elem_size, stride_bytes_256=sb256, gen_mode=0,
            single_packet=True, queue_num=queue_num, sbuf_tokens_per_rank=0,
            sbuf_free_dim_per_rank=0, sbuf_free_dim_pad_per_rank=0,
            sbuf_byte_offset=0))


def build(pl, n_nodes, cin, heads, hid, cout, n_cores):
    HC = heads * hid            # 256
    G1W = HC + heads            # 264 = [h | es] gather payload
    R1 = 384                    # hx row stride (768B)
    G2W = cout + 2              # 66  = [h2 | 1.0 | es2] gather payload
    R2 = 128                    # hx2 row stride (256B)
    npc, nblk = pl["npc"], pl["nblk"]
    KL, KH, Kb, tob = pl["KL"], pl["KH"], pl["Kb"], pl["tob"]
    Ttot, Kmax = pl["Ttot"], pl["Kmax"]
    NT1 = (n_nodes + P - 1) // P

    nc = bacc.Bacc("TRN2", num_swdge_queues=4)
    f32 = mybir.dt.float32
    bf16 = mybir.dt.bfloat16
    fp8 = mybir.dt.float8e4
    i16 = mybir.dt.int16
    Exp = mybir.ActivationFunctionType.Exp

    xT = nc.dram_tensor("xT", [cin, n_nodes], bf16, kind="ExternalInput")
    xTl = nc.dram_tensor("xTl", [cin, npc], bf16, kind="ExternalInput")
    Wx1 = nc.dram_tensor("Wx1", [cin, G1W], bf16, kind="ExternalInput")
    Wb1 = nc.dram_tensor("Wb1", [cin, heads], bf16, kind="ExternalInput")
    Wx2 = nc.dram_tensor("Wx2", [P, 2 * (cout + 2)], bf16, kind="ExternalInput")
    b1r = nc.dram_tensor("b1r", [P, HC], f32, kind="ExternalInput")
    b2r = nc.dram_tensor("b2r", [P, cout], f32, kind="ExternalInput")
    identb = nc.dram_tensor("identb", [P, P], bf16, kind="ExternalInput")
    srcw = nc.dram_tensor("srcw", [P, Ttot * 8], i16, kind="ExternalInput")
    mtc = nc.dram_tensor("mtc", [Ttot * P * P], fp8, kind="ExternalInput")
    iota_rep = nc.dram_tensor("iota_rep", [P, Kmax * P], bf16, kind="ExternalInput")
    slotb = nc.dram_tensor("slotb", [P, Ttot], bf16, kind="ExternalInput")
    out = nc.dram_tensor("out", [npc, cout], f32, kind="ExternalOutput")

    hx = nc.dram_tensor("hx", [n_nodes, R1], bf16)
    edt = nc.dram_tensor("edt", [npc, heads], bf16)
    edt2 = nc.dram_tensor("edt2", [npc, 1], bf16)
    hx2in = nc.dram_tensor("hx2in", [npc, R2], bf16)
    hx2 = nc.dram_tensor("hx2", [n_nodes, R2], bf16, addr_space="Shared")

    W2C = cout + 2              # 66 = [h2 | es2 | ed2] from the flush matmul
    Prelu = mybir.ActivationFunctionType.Prelu

    qctr = [0]

    def next_q():
        q = qctr[0] & 3
        qctr[0] += 1
        return q

    with tile.TileContext(nc) as tc:
        with tc.tile_pool(name="const", bufs=1) as cp:
            nc.gpsimd.load_library(library_config.mlp)
            ident_sb = cp.tile([P, P], bf16)
            b1_sb = cp.tile([P, HC], f32)
            b2_sb = cp.tile([P, cout], f32)
            w1_sb = cp.tile([cin, G1W], bf16)
            wb1_sb = cp.tile([cin, heads], bf16)
            w2_sb = cp.tile([P, 2 * W2C], bf16)
            sidx_sb = cp.tile([P, Ttot * 8], i16)
            iota_sb = cp.tile([P, Kmax, P], bf16)
            slot_sb = cp.tile([P, Ttot], bf16)
            nc.sync.dma_start(
                out=iota_sb[:, :, :],
                in_=iota_rep[:, :].rearrange("p (k q) -> p k q", q=P))
            nc.sync.dma_start(out=slot_sb[:], in_=slotb[:, :])
            nc.sync.dma_start(out=ident_sb[:], in_=identb[:, :])
            nc.sync.dma_start(out=b1_sb[:], in_=b1r[:, :])
            nc.sync.dma_start(out=b2_sb[:], in_=b2r[:, :])
            nc.sync.dma_start(out=w1_sb[:], in_=Wx1[:, :])
            nc.sync.dma_start(out=wb1_sb[:], in_=Wb1[:, :])
            nc.sync.dma_start(out=w2_sb[:], in_=Wx2[:, :])
            nc.sync.dma_start(out=sidx_sb[:], in_=srcw[:, :])

            # ---------- phase 1: hx = [x@W1 | es] bf16 for ALL nodes
            # (contiguous full-row stores); edt = x_local@Wb1 for LOCAL nodes
            XB = 8
            with tc.tile_pool(name="p1", bufs=3) as p1, \
                 tc.tile_pool(name="p1ps", bufs=4, space="PSUM") as p1ps:
                for j0 in range(0, NT1, XB):
                    jn = min(XB, NT1 - j0)
                    w = min(XB * P, n_nodes - j0 * P)
                    xt_sb = p1.tile([cin, XB * P], bf16, tag="xt")
                    nc.sync.dma_start(out=xt_sb[:, :w], in_=xT[:, j0 * P : j0 * P + w])
                    st = p1.tile([P, XB, R1], bf16, tag="st")
                    for k in range(jn):
                        rows = min(P, n_nodes - (j0 + k) * P)
                        ps = p1ps.tile([P, G1W], f32, tag="ps")
                        nc.tensor.matmul(out=ps[:rows, :],
                                         lhsT=xt_sb[:, k * P : k * P + rows],
                                         rhs=w1_sb[:, :], start=True, stop=True)
                        if k % 2 == 0:
                            nc.scalar.copy(out=st[:rows, k, 0:G1W], in_=ps[:rows, :])
                        else:
                            nc.vector.tensor_copy(out=st[:rows, k, 0:G1W], in_=ps[:rows, :])
                    full = jn if w == jn * P else jn - 1
                    if full:
                        nc.sync.dma_start(
                            out=hx[j0 * P : (j0 + full) * P, :].rearrange(
                                "(a b) c -> b a c", b=P),
                            in_=st[:, 0:full, :])
                    if full < jn:
                        rows = w - full * P
                        nc.sync.dma_start(
                            out=hx[(j0 + full) * P : j0 * P + w, 0:G1W],
                            in_=st[:rows, full, 0:G1W])
                for j0 in range(0, nblk, XB):
                    jn = min(XB, nblk - j0)
                    w = min(XB * P, npc - j0 * P)
                    xt_sb = p1.tile([cin, XB * P], bf16, tag="xt")
                    nc.sync.dma_start(out=xt_sb[:, :w], in_=xTl[:, j0 * P : j0 * P + w])
                    ste = p1.tile([P, XB, heads], bf16, tag="ste")
                    for k in range(jn):
                        rows = min(P, npc - (j0 + k) * P)
                        pse = p1ps.tile([P, heads], f32, tag="pse")
                        nc.tensor.matmul(out=pse[:rows, :],
                                         lhsT=xt_sb[:, k * P : k * P + rows],
                                         rhs=wb1_sb[:, :], start=True, stop=True)
                        if k % 2 == 0:
                            nc.scalar.copy(out=ste[:rows, k, :], in_=pse[:rows, :])
                        else:
                            nc.vector.tensor_copy(out=ste[:rows, k, :], in_=pse[:rows, :])
                    full = jn if w == jn * P else jn - 1
                    if full:
                        nc.sync.dma_start(
                            out=edt[j0 * P : (j0 + full) * P, :].rearrange(
                                "(a b) c -> b a c", b=P),
                            in_=ste[:, 0:full, :])
                    if full < jn:
                        rows = w - full * P
                        nc.sync.dma_start(
                            out=edt[(j0 + full) * P : j0 * P + w, :],
                            in_=ste[:rows, full, :])

            tc.strict_bb_all_engine_barrier()

            with tc.tile_pool(name="ps_acc", bufs=2, space="PSUM") as ps_acc, \
                 tc.tile_pool(name="ps_tr", bufs=2, space="PSUM") as ps_tr, \
                 tc.tile_pool(name="ps_small", bufs=2, space="PSUM") as ps_small, \
                 tc.tile_pool(name="ps_edp", bufs=2, space="PSUM") as ps_edp:

                def edge_layer(layer, table, RowW, gwidth, edtab, nheads,
                               chead, es_off, flush):
                    Cm = nheads * chead
                    Racc = Cm + nheads
                    lo_end = min(SPLIT, n_nodes)
                    with tc.tile_pool(name=f"eg{layer}", bufs=4) as eg, \
                         tc.tile_pool(name=f"ew{layer}", bufs=3) as ew, \
                         tc.tile_pool(name=f"em{layer}", bufs=3) as em:
                        for b in range(nblk):
                            kl, kh = int(KL[b]), int(KH[b])
                            K = kl + kh
                            t0 = int(tob[b])
                            rows = min(P, npc - b * P)
                            # per-block e_dst rows: one contiguous DMA
                            edr = ew.tile([P, nheads], bf16, tag="edr")
                            if rows < P:
                                nc.vector.memset(edr[:, :], 0.0)
                            nc.sync.dma_start(
                                out=edr[:rows, :],
                                in_=edtab[b * P : b * P + rows, :])
                            # mtT one-hot: one sequential read; mt via is_eq
                            mtT = em.tile([P, Kmax * P], fp8, tag="mtT")
                            o0 = t0 * P * P
                            nc.sync.dma_start(
                                out=mtT[:, 0:K * P],
                                in_=mtc[o0:o0 + K * P * P].rearrange(
                                    "(p c) -> p c", p=P))
                            mt = em.tile([P, Kmax, P], bf16, tag="mt")
                            nc.vector.tensor_tensor(
                                out=mt[:, 0:K, :], in0=iota_sb[:, 0:K, :],
                                in1=slot_sb[:, t0:t0 + K].to_broadcast(
                                    [P, K, P]),
                                op=AluOpType.is_equal)
                            hxg_t = eg.tile([P, Kmax, gwidth], bf16, tag="hxg")
                            hxg = hxg_t[:, 0:K, :]
                            # queue 0 blocks the engine while emitting, so it
                            # gets a single 1-group chunk; the bulk splits
                            # evenly over the async queues 1-3.
                            lo_tab = table[0:lo_end, 0:gwidth]
                            hi_tab = table[SPLIT:n_nodes, 0:gwidth]
                            segs = [(0, kl, lo_tab), (kl, kh, hi_tab)]
                            chunks = []   # (c0, ncc, tab)
                            first = next((i for i, s in enumerate(segs) if s[1] > 0), None)
                            for i, (base, n, tab) in enumerate(segs):
                                if n == 0:
                                    continue
                                c0 = base
                                if i == first:
                                    chunks.append((c0, 1, tab, 0))
                                    c0 += 1
                                    n -= 1
                                parts = (n + GMAX - 1) // GMAX
                                if n > 0:
                                    parts = max(parts, min(n, 3 if i == first else 2))
                                for j in range(parts):
                                    ncc = n // parts + (1 if j < n % parts else 0)
                                    if ncc == 0:
                                        continue
                                    chunks.append((c0, ncc, tab, None))
                                    c0 += ncc
                            for c0, ncc, tab, qfix in chunks:
                                q = qfix if qfix is not None else (qctr[0] % 3) + 1
                                if qfix is None:
                                    qctr[0] += 1
                                raw_dma_gather(
                                    nc, hxg[:, c0:c0 + ncc, :], tab,
                                    sidx_sb[:, (t0 + c0) * 8:(t0 + c0 + ncc) * 8],
                                    ncc * P, gwidth, RowW, q)
                            # e_dst per edge: edp[p, k*nh:] = edr[slot[p,k], :]
                            edp = ps_edp.tile([P, Kmax * nheads], f32,
                                              tag="edp", space="PSUM")
                            for k in range(K):
                                nc.tensor.matmul(
                                    out=edp[:, k * nheads:(k + 1) * nheads],
                                    lhsT=mtT[:, k * P:(k + 1) * P],
                                    rhs=edr[:, :], start=True, stop=True)
                            tmp = ew.tile([P, K, nheads], f32, tag="tmp")
                            nc.vector.tensor_tensor(
                                out=tmp[:, :, :],
                                in0=hxg[:, :, es_off : es_off + nheads],
                                in1=edp[:, 0:K * nheads].rearrange(
                                    "p (k h) -> p k h", k=K), op=AluOpType.add)
                            # exp(leaky_relu(.)) on the scalar engine
                            nc.scalar.activation(tmp[:, :, :], tmp[:, :, :],
                                                 Prelu, alpha=NEG_SLOPE)
                            ex = ew.tile([P, K, nheads], bf16, tag="ex")
                            nc.scalar.activation(ex[:, :, :], tmp[:, :, :], Exp)
                            rhs = ew.tile([P, K, Racc], bf16, tag="rhs")
                            if nheads == 1:
                                # table row is [h | 1.0 | es]: one multiply
                                # yields [h*ex | ex] including the denominator
                                nc.vector.tensor_tensor(
                                    out=rhs[:, :, :],
                                    in0=hxg[:, :, 0:Racc],
                                    in1=ex[:, :, 0:1].to_broadcast([P, K, Racc]),
                                    op=AluOpType.mult)
                            else:
                                nc.vector.tensor_tensor(
                                    out=rhs[:, :, 0:Cm].rearrange(
                                        "p k (h c) -> p k h c", h=nheads),
                                    in0=hxg[:, :, 0:Cm].rearrange(
                                        "p k (h c) -> p k h c", h=nheads),
                                    in1=ex[:, :, :].to_broadcast(
                                        [P, K, nheads, chead]),
                                    op=AluOpType.mult)
                                nc.vector.tensor_copy(out=rhs[:, :, Cm:Racc],
                                                      in_=ex[:, :, :])
                            acc = ps_acc.tile([P, Racc], f32, tag="acc", space="PSUM")
                            for k in range(K):
                                nc.tensor.matmul(out=acc[:, :],
                                                 lhsT=mt[:, k, :],
                                                 rhs=rhs[:, k, :],
                                                 start=(k == 0), stop=(k == K - 1))
                            flush(b, acc, rows, ew)

                # ----- layer 1 flush: normalize, elu, layer-2 dense, store
                def flush1(b, acc, rows, ew):
                    recip = ew.tile([P, heads], f32, tag="recip")
                    nc.vector.reciprocal(recip[:rows, :], acc[:rows, HC : HC + heads])
                    h1 = ew.tile([P, HC], f32, tag="h1")
                    nc.vector.tensor_tensor(
                        out=h1[:rows, :].rearrange("p (h c) -> p h c", h=heads),
                        in0=acc[:rows, 0:HC].rearrange("p (h c) -> p h c", h=heads),
                        in1=recip[:rows, :].to_broadcast([rows, heads, hid]),
                        op=AluOpType.mult)
                    nc.vector.tensor_add(out=h1[:rows, :], in0=h1[:rows, :],
                                         in1=b1_sb[:rows, :])
                    mn = ew.tile([P, HC], f32, tag="mn")
                    nc.vector.tensor_scalar_min(mn[:rows, :], h1[:rows, :], 0.0)
                    nc.scalar.activation(mn[:rows, :], mn[:rows, :], Exp)
                    mx = ew.tile([P, HC], f32, tag="mx")
                    nc.vector.tensor_scalar_max(mx[:rows, :], h1[:rows, :], 0.0)
                    h1e = ew.tile([P, HC], bf16, tag="h1e")
                    nc.vector.scalar_tensor_tensor(
                        out=h1e[:rows, :], in0=mn[:rows, :], scalar=-1.0,
                        in1=mx[:rows, :], op0=AluOpType.add, op1=AluOpType.add)
                    tp = ew.tile([P, 2, P], bf16, tag="tp")
                    h2p = ps_small.tile([P, W2C], f32, tag="h2p", space="PSUM")
                    for ch in range(2):
                        tps = ps_tr.tile([P, P], bf16, tag="trp", space="PSUM")
                        nc.tensor.transpose(out=tps[:], in_=h1e[:, ch * P:(ch + 1) * P],
                                            identity=ident_sb[:])
                        nc.scalar.copy(out=tp[:, ch, :], in_=tps[:])
                        nc.tensor.matmul(
                            out=h2p[:rows, :],
                            lhsT=tp[:, ch, 0:rows],
                            rhs=w2_sb[:, ch * W2C:(ch + 1) * W2C],
                            start=(ch == 0), stop=(ch == 1))
                    st2 = ew.tile([P, R2], bf16, tag="st2")
                    nc.vector.memset(st2[:rows, G2W:R2], 0.0)
                    nc.vector.memset(st2[:rows, cout:cout + 1], 1.0)
                    nc.scalar.copy(out=st2[:rows, 0:cout], in_=h2p[:rows, 0:cout])
                    nc.scalar.copy(out=st2[:rows, cout + 1:cout + 2],
                                   in_=h2p[:rows, cout:cout + 1])
                    nc.sync.dma_start(out=hx2in[b * P : b * P + rows, :],
                                      in_=st2[:rows, :])
                    se2 = ew.tile([P, 1], bf16, tag="se2")
                    nc.scalar.copy(out=se2[:rows, :],
                                   in_=h2p[:rows, cout + 1:cout + 2])
                    nc.sync.dma_start(out=edt2[b * P : b * P + rows, :],
                                      in_=se2[:rows, :])

                edge_layer(1, hx, R1, G1W, edt, heads, hid, HC, flush1)

                # ----- AllGather hx2 (issued without a pre-barrier; Tile
                # orders it after the hx2in stores via data deps)
                nc.gpsimd.collective_compute(
                    "AllGather", AluOpType.bypass,
                    replica_groups=[list(range(n_cores))],
                    ins=[hx2in[:, :]], outs=[hx2[:, :]])
                tc.strict_bb_all_engine_barrier()

                # ----- layer-2 edge pass (gathers straight from Shared hx2)
                def flush2(b, acc, rows, ew):
                    recip = ew.tile([P, 1], f32, tag="recip2")
                    nc.vector.reciprocal(recip[:rows, :], acc[:rows, cout : cout + 1])
                    o = ew.tile([P, cout], f32, tag="o")
                    nc.vector.tensor_tensor(
                        out=o[:rows, :], in0=acc[:rows, 0:cout],
                        in1=recip[:rows, :].to_broadcast([rows, cout]),
                        op=AluOpType.mult)
                    nc.vector.tensor_add(out=o[:rows, :], in0=o[:rows, :],
                                         in1=b2_sb[:rows, :])
                    nc.sync.dma_start(out=out[b * P : b * P + rows, :],
                                      in_=o[:rows, :])

                edge_layer(2, hx2, R2, G2W, edt2, 1, cout, cout + 1, flush2)

    nc.compile()
    return nc


def host_prep(x, edge_index, W1, a_src1, a_dst1, b1, W2, a_src2, a_dst2, b2,
              n_cores):
    """Plan the edge partition and build per-core input maps."""
    x = np.asarray(x, np.float32)
    n_nodes, cin = x.shape
    heads, hid = np.asarray(a_src1).shape
    cout = np.asarray(W2).shape[1]
    npc = n_nodes // n_cores

    loops = np.arange(n_nodes, dtype=np.int64)
    src = np.concatenate([np.asarray(edge_index[0], np.int64), loops])
    dst = np.concatenate([np.asarray(edge_index[1], np.int64), loops])
    pl = plan(src, dst, n_nodes, n_cores)

    W1 = np.asarray(W1, np.float32)
    W1h = W1.reshape(cin, heads, hid)
    Wa1 = np.einsum("khc,hc->kh", W1h, np.asarray(a_src1, np.float32))
    Wb1 = np.einsum("khc,hc->kh", W1h, np.asarray(a_dst1, np.float32))
    Wx1 = np.concatenate([W1, Wa1], axis=1).astype(BF)

    W2 = np.asarray(W2, np.float32)
    Wa2 = (W2 * np.asarray(a_src2, np.float32)).sum(1, keepdims=True)
    Wb2 = (W2 * np.asarray(a_dst2, np.float32)).sum(1, keepdims=True)
    W2e = np.concatenate([W2, Wa2, Wb2], axis=1)
    Wx2 = np.ascontiguousarray(
        np.concatenate([W2e[:P], W2e[P:]], axis=1)).astype(BF)

    xTb = np.ascontiguousarray(x.T).astype(BF)
    common = {
        "xT": xTb,
        "Wx1": Wx1,
        "Wb1": np.ascontiguousarray(Wb1).astype(BF),
        "Wx2": Wx2,
        "b1r": np.tile(np.asarray(b1, np.float32)[None, :], (P, 1)),
        "b2r": np.tile(np.asarray(b2, np.float32)[None, :], (P, 1)),
        "identb": np.eye(P, dtype=np.float32).astype(BF),
        "iota_rep": np.tile(np.arange(P, dtype=np.float32),
                            (P, pl["Kmax"])).astype(BF),
    }
    in_maps = []
    for c in range(n_cores):
        m = dict(common)
        m["xTl"] = np.ascontiguousarray(xTb[:, c * npc:(c + 1) * npc])
        m["srcw"] = np.ascontiguousarray(pl["srcw"][c])
        m["mtc"] = np.ascontiguousarray(pl["mtc"][c])
        m["slotb"] = np.ascontiguousarray(pl["slot"][c]).astype(BF)
        in_maps.append(m)
    return pl, (n_nodes, cin, heads, hid, cout), in_maps


def run_gat(x, edge_index, W1, a_src1, a_dst1, b1, W2, a_src2, a_dst2, b2,
            n_cores=8, trace=False):
    pl, (n_nodes, cin, heads, hid, cout), in_maps = host_prep(
        x, edge_index, W1, a_src1, a_dst1, b1, W2, a_src2, a_dst2, b2, n_cores)
    nc = build(pl, n_nodes, cin, heads, hid, cout, n_cores)
    res = bass_utils.run_bass_kernel_spmd(
        nc, in_maps, core_ids=list(range(n_cores)), trace=trace)
    outp = np.concatenate([res.results[c]["out"] for c in range(n_cores)], axis=0)
    return outp[:n_nodes], res


def kernel(**inputs):
    """Full-input GAT kernel: shards internally across 8 NeuronCores."""
    x = np.asarray(inputs["x"], np.float32)
    edge_index = np.asarray(inputs["edge_index"])
    outp, _ = run_gat(
        x, edge_index,
        inputs["W1"], inputs["a_src1"], inputs["a_dst1"], inputs["b1"],
        inputs["W2"], inputs["a_src2"], inputs["a_dst2"], inputs["b2"],
        n_cores=8, trace=bool(int(os.environ.get("GAT_TRACE", "0"))))
    return outp.astype(np.float32)
